# revision 12
# baseline (speedup 1.0000x reference)
"""Causal single-head attention on 8 trn2 NeuronCores — single SPMD program.

Problem: x[4,4096,768], WQ/WK/WV[768,64] -> out[4,4096,64]
  Q=x@WQ K=x@WK V=x@WV; causal softmax(QK^T/8)@V per batch.

Sharding: core c handles batch b=c//2 with query-interleave h=c%2 (its
queries are global rows h, h+2, h+4, ... of the batch).  Interleaving
makes the causal structure IDENTICAL on all 8 cores: local q-block j
(512 queries spanning global rows 1024j+h .. 1024j+1022+h) always sees
k-chunks 0..8j+7.  The only h-dependence is a +-1 shift of the diagonal
mask boundary, which is supplied as a tiny per-core 0/1 mask INPUT and
applied with one vector multiply after exp — so one program serves all
cores and the whole forward pass is a single 8-device dispatch.

Host staging (outside the kernel): x is pre-transposed per core
(xt = x[b].T for K/V, xqt = x[b][h::2].T for Q), which removes all PE
transposes of x from the device kernel.

Kernel per core (all fp32):
  For each s-block sb (512 keys): DMA xT tile, project [K^T|V^T] with
  fused [WK|WV] stationary; build Vplus=[V|1] chunks via PE transpose.
  On even sb: project Q^T for q-block j=sb//2.  After odd sb: flash
  attention for q-block j=sb//2 with scores TRANSPOSED (keys on
  partitions): scoresT[k,q] = matmul(KT chunk, QT block); exp on ACT
  (scale=1/8, no max subtraction — scores ~ N(0,1), safe in fp32);
  causal zeroing on the 8 diagonal chunks via mask multiply;
  OT[65,q] += matmul(Vplus[128,65], PT) — row 64 = softmax denominator.
  Epilogue: PE-transpose OT -> O natural, reciprocal * scale, DMA out.
"""
import sys
import os

sys.path.insert(0, "/opt/trn_rl_repo")

import numpy as np

B, S, DM, DK = 4, 4096, 768, 64
NCORE = 8
SQ = S // 2          # queries per core (interleaved)
NSB = S // 512       # 8 s-blocks of 512 keys
NQB = SQ // 512      # 4 local q-blocks of 512 queries

_cache = {}


def _split_waits(nc, mybir, maxw=1):
    """Walrus here accepts only 1 sem-wait per instruction; move excess
    waits onto preceding same-engine no-ops."""
    cnt = 0
    for bb in nc.m.functions[0].blocks:
        new_insts = []
        for inst in bb.instructions:
            si = inst.sync_info
            if si is not None and si.on_wait and len(si.on_wait) > maxw:
                waits = list(si.on_wait)
                si.on_wait = waits[:maxw]
                extra = waits[maxw:]
                for i in range(0, len(extra), maxw):
                    cnt += 1
                    nop = mybir.InstNoOp(name=f"waitsplit-{cnt}", ins=[], outs=[])
                    nop.engine = inst.engine
                    nop.sync_info = mybir.SyncInfo(
                        on_wait=extra[i : i + maxw], on_update=[]
                    )
                    new_insts.append(nop)
            new_insts.append(inst)
        bb.instructions[:] = new_insts


def _build_program():
    import concourse.bass as bass
    import concourse.mybir as mybir
    from concourse.tile import TileContext
    from concourse.masks import make_identity

    f32 = mybir.dt.float32
    bf16 = mybir.dt.bfloat16
    AF = mybir.ActivationFunctionType

    nc = bass.Bass()
    xt = nc.declare_dram_parameter("xt", [DM, S], bf16, isOutput=False)
    xqt = nc.declare_dram_parameter("xqt", [DM, SQ], bf16, isOutput=False)
    wq = nc.declare_dram_parameter("wq", [DM, DK], bf16, isOutput=False)
    wk = nc.declare_dram_parameter("wk", [DM, DK], bf16, isOutput=False)
    wv = nc.declare_dram_parameter("wv", [DM, DK], bf16, isOutput=False)
    cmp_ = nc.declare_dram_parameter("cmp", [128, 512], f32, isOutput=False)
    out = nc.declare_dram_parameter("out", [SQ, DK], f32, isOutput=True)

    with TileContext(nc) as tc:
        with (
            tc.tile_pool(name="consts", bufs=1) as cpool,
            tc.tile_pool(name="big", bufs=1) as big,
        ):
            ident = cpool.tile([128, 128], f32)
            make_identity(nc, ident[:])
            identb = cpool.tile([128, 128], bf16)
            nc.vector.tensor_copy(identb[:], ident[:])
            # [WK|WV] stationary chunks: cols 0:64 = WK, 64:128 = WV.
            # One strided DMA per weight tensor (dest viewed [128, 6, .]).
            wkv = cpool.tile([128, 6, 128], bf16)
            wqt = cpool.tile([128, 6 * 64], bf16)
            nc.sync.dma_start(
                wkv[:, :, 0:64], wk[:].rearrange("(c p) d -> p c d", p=128)
            )
            nc.sync.dma_start(
                wkv[:, :, 64:128], wv[:].rearrange("(c p) d -> p c d", p=128)
            )
            nc.sync.dma_start(
                wqt[:].rearrange("p (c d) -> p c d", d=64),
                wq[:].rearrange("(c p) d -> p c d", p=128),
            )
            cmpt = cpool.tile([128, 512], f32)
            nc.sync.dma_start(cmpt[:], cmp_[:])

            KTVT = big.tile([128, S], bf16)  # rows 0:64 = K^T, 64:128 = V^T
            QT = big.tile([64, SQ], bf16)  # own queries, transposed
            VP = big.tile([128, 32 * 65], bf16)  # [V|1] per k-chunk
            VPr = VP[:].rearrange("p (c u) -> p c u", u=65)
            nc.gpsimd.memset(VP[:], 1.0)  # ones column; V part overwritten

            # PSUM budget (8 banks): kvqt 2 + tp 2 + st 2 + ot 2
            with (
                tc.tile_pool(name="xload", bufs=1) as xl_pool,
                tc.tile_pool(name="kv_ps", bufs=2, space="PSUM") as kv_psum,
                tc.tile_pool(name="tp_ps", bufs=1, space="PSUM") as tp_psum,
                tc.tile_pool(name="s_ps", bufs=3, space="PSUM") as s_psum,
                tc.tile_pool(name="ot_ps", bufs=2, space="PSUM") as ot_psum,
                tc.tile_pool(name="pt", bufs=6) as pt_pool,
                tc.tile_pool(name="ep", bufs=2) as ep_pool,
            ):
                reps = int(os.environ.get("K_REPS", "1"))

                def emit_epilogue(j, ot):
                    ots = ep_pool.tile([65, 512], f32, name="ots", tag="ots")
                    nc.vector.tensor_copy(ots[:], ot[:])
                    o_n = ep_pool.tile([128, 4, 65], f32, name="o_n", tag="on")
                    rec = ep_pool.tile([128, 4], f32, name="rec", tag="rc")
                    for u in range(4):
                        tp2 = s_psum.tile([128, 512], f32, name="tp2", tag="st")
                        nc.tensor.transpose(
                            tp2[:, 0:65],
                            ots[:, u * 128 : (u + 1) * 128],
                            ident[0:65, 0:65],
                        )
                        nc.vector.tensor_copy(o_n[:, u, :], tp2[:, 0:65])
                        nc.vector.reciprocal(rec[:, u : u + 1], o_n[:, u, 64:65])
                        nc.vector.tensor_scalar_mul(
                            o_n[:, u, 0:64], o_n[:, u, 0:64], rec[:, u : u + 1]
                        )
                    ov = out[j * 512 : (j + 1) * 512, :].rearrange(
                        "(u p) d -> p u d", p=128
                    )
                    nc.sync.dma_start(ov, o_n[:, :, 0:64])

                for _rep in range(reps):
                    # Front-loaded input staging: one DMA per (d-chunk,
                    # block), issued in usage order — spreads the load
                    # across all DMA queues so the first blocks land fast
                    # and every block has its own fine-grained dependency.
                    xall = xl_pool.tile([128, 6, NSB * 512], bf16, tag="xa")
                    xqall = xl_pool.tile([128, 6, NQB * 512], bf16, tag="xq")
                    for sb in range(NSB):
                        for c in range(6):
                            nc.sync.dma_start(
                                xall[:, c, sb * 512 : (sb + 1) * 512],
                                xt[c * 128 : (c + 1) * 128,
                                   sb * 512 : (sb + 1) * 512],
                            )
                        if sb % 2 == 0:
                            j = sb // 2
                            for c in range(6):
                                nc.sync.dma_start(
                                    xqall[:, c, j * 512 : (j + 1) * 512],
                                    xqt[c * 128 : (c + 1) * 128,
                                        j * 512 : (j + 1) * 512],
                                )
                    for sb in range(NSB):
                        # ---- K/V projection for this s-block ----
                        pass
                        kv = kv_psum.tile([128, 512], f32, name="kv", tag="kvqt")
                        for c in range(6):
                            nc.tensor.matmul(
                                kv[:],
                                wkv[:, c, :],
                                xall[:, c, sb * 512 : (sb + 1) * 512],
                                start=(c == 0),
                                stop=(c == 5),
                            )
                        nc.vector.tensor_copy(
                            KTVT[:, sb * 512 : (sb + 1) * 512], kv[:]
                        )
                        # Vplus chunks for this s-block (V natural layout)
                        for u in range(4):
                            kc = sb * 4 + u
                            vtp = tp_psum.tile([128, 64], bf16, name="vtp", tag="tp")
                            nc.tensor.transpose(
                                vtp[:, :],
                                KTVT[64:128, kc * 128 : (kc + 1) * 128],
                                identb[64:128, 64:128],
                            )
                            nc.vector.tensor_copy(VPr[:, kc, 0:64], vtp[:, :])
                        if sb % 2 == 0:
                            # ---- Q projection for q-block j = sb//2 ----
                            j = sb // 2
                            qt = kv_psum.tile([64, 512], f32, name="qt", tag="kvqt")
                            for c in range(6):
                                nc.tensor.matmul(
                                    qt[:],
                                    wqt[:, c * 64 : (c + 1) * 64],
                                    xqall[:, c, j * 512 : (j + 1) * 512],
                                    start=(c == 0),
                                    stop=(c == 5),
                                )
                            nc.vector.tensor_copy(
                                QT[:, j * 512 : (j + 1) * 512], qt[:]
                            )
                        else:
                            # ---- attention for q-block j = sb//2 ----
                            # Software-pipelined: scores/exp run L chunks
                            # ahead of the PV accumulation so the PE never
                            # waits on the ACT->mask chain.
                            j = sb // 2
                            nkc = 8 * j + 8
                            L = 2
                            ot = ot_psum.tile([65, 512], f32, name="ot", tag="ot")
                            pts = {}
                            for kc in range(nkc + L):
                                if kc < nkc:
                                    st = s_psum.tile(
                                        [128, 512], f32, name="st", tag="st"
                                    )
                                    nc.tensor.matmul(
                                        st[:],
                                        KTVT[0:64, kc * 128 : (kc + 1) * 128],
                                        QT[:, j * 512 : (j + 1) * 512],
                                        start=True,
                                        stop=True,
                                    )
                                    pt = pt_pool.tile(
                                        [128, 512], bf16, name="pt", tag="pt"
                                    )
                                    nc.scalar.activation(
                                        pt[:], st[:], AF.Exp, scale=0.125
                                    )
                                    d = kc - 8 * j
                                    if d >= 0:
                                        # pt *= (C >= 128d): causal zeroing
                                        nc.vector.scalar_tensor_tensor(
                                            pt[:],
                                            cmpt[:],
                                            float(128 * d),
                                            pt[:],
                                            op0=mybir.AluOpType.is_ge,
                                            op1=mybir.AluOpType.mult,
                                        )
                                    pts[kc] = pt
                                if kc >= L:
                                    nc.tensor.matmul(
                                        ot[:],
                                        VPr[:, kc - L, :],
                                        pts.pop(kc - L)[:],
                                        start=(kc - L == 0),
                                        stop=(kc - L == nkc - 1),
                                    )
                            emit_epilogue(j, ot)

    import concourse.mybir as mybir
    _split_waits(nc, mybir)
    return nc


def _make_runner(nc, n_cores, dev_offset):
    """Compile to a jitted shard_map callable over an explicit device subset."""
    import jax
    import concourse.mybir as mybir
    from concourse import bass2jax
    from jax.experimental.shard_map import shard_map
    from jax.sharding import Mesh, PartitionSpec, NamedSharding

    bass2jax.install_neuronx_cc_hook()

    partition_name = (
        nc.partition_id_tensor.name if nc.partition_id_tensor else None
    )
    in_names, out_names, out_avals, zero_outs = [], [], [], []
    for alloc in nc.m.functions[0].allocations:
        if not isinstance(alloc, mybir.MemoryLocationSet):
            continue
        name = alloc.memorylocations[0].name
        if alloc.kind == "ExternalInput":
            if name != partition_name:
                in_names.append(name)
        elif alloc.kind == "ExternalOutput":
            shape = tuple(alloc.tensor_shape)
            dtype = mybir.dt.np(alloc.dtype)
            out_avals.append(jax.core.ShapedArray(shape, dtype))
            out_names.append(name)
            zero_outs.append(np.zeros(shape, dtype))
    n_params = len(in_names)
    n_outs = len(out_avals)
    all_names = in_names + out_names
    if partition_name is not None:
        all_names.append(partition_name)

    def _body(*args):
        operands = list(args)
        if partition_name is not None:
            operands.append(bass2jax.partition_id_tensor())
        outs = bass2jax._bass_exec_p.bind(
            *operands,
            out_avals=tuple(out_avals),
            in_names=tuple(all_names),
            out_names=tuple(out_names),
            lowering_input_output_aliases=(),
            sim_require_finite=True,
            sim_require_nnan=True,
            nc=nc,
        )
        return tuple(outs)

    devices = jax.devices()[dev_offset : dev_offset + n_cores]
    mesh = Mesh(np.asarray(devices), ("core",))
    in_specs = (PartitionSpec("core"),) * (n_params + n_outs)
    out_specs = (PartitionSpec("core"),) * n_outs
    sharded = jax.jit(
        shard_map(
            _body, mesh=mesh, in_specs=in_specs, out_specs=out_specs, check_rep=False
        ),
        keep_unused=True,
    )
    sh = NamedSharding(mesh, PartitionSpec("core"))

    def prepare(in_maps):
        per_core = [[np.asarray(m[n]) for n in in_names] for m in in_maps]
        concat_in = [
            np.concatenate([per_core[c][i] for c in range(n_cores)], axis=0)
            for i in range(n_params)
        ]
        concat_zeros = [
            np.zeros((n_cores * z.shape[0], *z.shape[1:]), z.dtype)
            for z in zero_outs
        ]
        return [jax.device_put(a, sh) for a in concat_in + concat_zeros]

    def run(in_maps):
        return sharded(*prepare(in_maps))

    run.sharded = sharded
    run.prepare = prepare
    run.out_names = out_names
    run.out_avals = out_avals
    run.n_cores = n_cores
    return run


def _bf16():
    import ml_dtypes

    return ml_dtypes.bfloat16


def _make_cmp(h):
    """Causal comparison base for interleave offset h.

    C[p, i] = 2i + h - p.  Key 128*(8j+d)+p is visible to local query
    1024j+2i+h iff C[p, i] >= 128d (j-independent), applied on-device as
    pt *= (C >= 128d) for the 8 diagonal chunks d."""
    p = np.arange(128)[:, None]
    i = np.arange(512)[None, :]
    return (2 * i + h - p).astype(np.float32)


def _get_runner():
    if "runner" not in _cache:
        nc = _build_program()
        _cache["runner"] = _make_runner(nc, NCORE, 0)
    return _cache["runner"]


def _core_maps(x, WQ, WK, WV):
    bf = _bf16()
    cmps = [_make_cmp(0), _make_cmp(1)]
    maps = []
    for c in range(NCORE):
        b, h = c // 2, c % 2
        maps.append(
            {
                "xt": np.ascontiguousarray(x[b].T).astype(bf),
                "xqt": np.ascontiguousarray(x[b][h::2].T).astype(bf),
                "wq": WQ.astype(bf),
                "wk": WK.astype(bf),
                "wv": WV.astype(bf),
                "cmp": cmps[h],
            }
        )
    return maps


def kernel(x, WQ, WK, WV):
    run = _get_runner()
    res = run(_core_maps(x, WQ, WK, WV))
    halves = np.asarray(res[0]).reshape(NCORE, SQ, DK)
    out = np.empty((B, S, DK), np.float32)
    for c in range(NCORE):
        b, h = c // 2, c % 2
        out[b, h::2] = halves[c]
    return out


if __name__ == "__main__":
    rng = np.random.default_rng(0)
    x = rng.standard_normal((B, S, DM), dtype=np.float32)
    sc = 1.0 / np.sqrt(DM)
    WQ = rng.standard_normal((DM, DK), dtype=np.float32) * sc
    WK = rng.standard_normal((DM, DK), dtype=np.float32) * sc
    WV = rng.standard_normal((DM, DK), dtype=np.float32) * sc
    got = kernel(x, WQ, WK, WV)
    # numpy reference
    Q = x @ WQ
    K = x @ WK
    V = x @ WV
    sref = np.einsum("bqd,bkd->bqk", Q, K) / 8.0
    mask = np.tril(np.ones((S, S), bool))
    sref = np.where(mask, sref, -np.inf)
    sref = sref - sref.max(-1, keepdims=True)
    p = np.exp(sref)
    p /= p.sum(-1, keepdims=True)
    ref = np.einsum("bqk,bkv->bqv", p, V)
    err = np.abs(got - ref).max() / np.abs(ref).max()
    print("rel err:", err)


# revision 13
# speedup vs baseline: 1.2164x; 1.2164x over previous
"""Causal single-head attention on 8 trn2 NeuronCores — single SPMD program.

Problem: x[4,4096,768], WQ/WK/WV[768,64] -> out[4,4096,64]
  Q=x@WQ K=x@WK V=x@WV; causal softmax(QK^T/8)@V per batch.

Sharding: core c handles batch b=c//2 with query-interleave h=c%2 (its
queries are global rows h, h+2, h+4, ... of the batch).  Interleaving
makes the causal structure IDENTICAL on all 8 cores: local q-block j
(512 queries spanning global rows 1024j+h .. 1024j+1022+h) always sees
k-chunks 0..8j+7.  The only h-dependence is a +-1 shift of the diagonal
mask boundary, which is supplied as a tiny per-core 0/1 mask INPUT and
applied with one vector multiply after exp — so one program serves all
cores and the whole forward pass is a single 8-device dispatch.

Host staging (outside the kernel): x is pre-transposed per core
(xt = x[b].T for K/V, xqt = x[b][h::2].T for Q), which removes all PE
transposes of x from the device kernel.

Kernel per core (all fp32):
  For each s-block sb (512 keys): DMA xT tile, project [K^T|V^T] with
  fused [WK|WV] stationary; build Vplus=[V|1] chunks via PE transpose.
  On even sb: project Q^T for q-block j=sb//2.  After odd sb: flash
  attention for q-block j=sb//2 with scores TRANSPOSED (keys on
  partitions): scoresT[k,q] = matmul(KT chunk, QT block); exp on ACT
  (scale=1/8, no max subtraction — scores ~ N(0,1), safe in fp32);
  causal zeroing on the 8 diagonal chunks via mask multiply;
  OT[65,q] += matmul(Vplus[128,65], PT) — row 64 = softmax denominator.
  Epilogue: PE-transpose OT -> O natural, reciprocal * scale, DMA out.
"""
import sys
import os

sys.path.insert(0, "/opt/trn_rl_repo")

import numpy as np

B, S, DM, DK = 4, 4096, 768, 64
NCORE = 8
SQ = S // 2          # queries per core (interleaved)
NSB = S // 512       # 8 s-blocks of 512 keys
NQB = SQ // 512      # 4 local q-blocks of 512 queries

_cache = {}


def _split_waits(nc, mybir, maxw=1):
    """Walrus here accepts only 1 sem-wait per instruction; move excess
    waits onto preceding same-engine no-ops."""
    cnt = 0
    for bb in nc.m.functions[0].blocks:
        new_insts = []
        for inst in bb.instructions:
            si = inst.sync_info
            if si is not None and si.on_wait and len(si.on_wait) > maxw:
                waits = list(si.on_wait)
                si.on_wait = waits[:maxw]
                extra = waits[maxw:]
                for i in range(0, len(extra), maxw):
                    cnt += 1
                    nop = mybir.InstNoOp(name=f"waitsplit-{cnt}", ins=[], outs=[])
                    nop.engine = inst.engine
                    nop.sync_info = mybir.SyncInfo(
                        on_wait=extra[i : i + maxw], on_update=[]
                    )
                    new_insts.append(nop)
            new_insts.append(inst)
        bb.instructions[:] = new_insts


def _build_program():
    import concourse.bass as bass
    import concourse.mybir as mybir
    from concourse.tile import TileContext
    from concourse.masks import make_identity

    f32 = mybir.dt.float32
    bf16 = mybir.dt.bfloat16
    AF = mybir.ActivationFunctionType

    nc = bass.Bass()
    xt = nc.declare_dram_parameter("xt", [DM, S], bf16, isOutput=False)
    xqt = nc.declare_dram_parameter("xqt", [DM, SQ], bf16, isOutput=False)
    wq = nc.declare_dram_parameter("wq", [DM, DK], bf16, isOutput=False)
    wk = nc.declare_dram_parameter("wk", [DM, DK], bf16, isOutput=False)
    wv = nc.declare_dram_parameter("wv", [DM, DK], bf16, isOutput=False)
    cmp_ = nc.declare_dram_parameter("cmp", [128, 512], f32, isOutput=False)
    out = nc.declare_dram_parameter("out", [SQ, DK], f32, isOutput=True)

    with TileContext(nc) as tc:
        with (
            tc.tile_pool(name="consts", bufs=1) as cpool,
            tc.tile_pool(name="big", bufs=1) as big,
        ):
            ident = cpool.tile([128, 128], f32)
            make_identity(nc, ident[:])
            identb = cpool.tile([128, 128], bf16)
            nc.vector.tensor_copy(identb[:], ident[:])
            # [WK|WV] stationary chunks: cols 0:64 = WK, 64:128 = WV.
            # One strided DMA per weight tensor (dest viewed [128, 6, .]).
            wkv = cpool.tile([128, 6, 128], bf16)
            wqt = cpool.tile([128, 6 * 64], bf16)
            nc.sync.dma_start(
                wkv[:, :, 0:64], wk[:].rearrange("(c p) d -> p c d", p=128)
            )
            nc.sync.dma_start(
                wkv[:, :, 64:128], wv[:].rearrange("(c p) d -> p c d", p=128)
            )
            nc.sync.dma_start(
                wqt[:].rearrange("p (c d) -> p c d", d=64),
                wq[:].rearrange("(c p) d -> p c d", p=128),
            )
            cmpt = cpool.tile([128, 512], f32)
            nc.sync.dma_start(cmpt[:], cmp_[:])

            KTVT = big.tile([128, S], bf16)  # rows 0:64 = K^T, 64:128 = V^T
            QT = big.tile([64, SQ], bf16)  # own queries, transposed
            VP = big.tile([128, 32 * 65], bf16)  # [V|1] per k-chunk
            VPr = VP[:].rearrange("p (c u) -> p c u", u=65)
            nc.gpsimd.memset(VP[:], 1.0)  # ones column; V part overwritten

            # PSUM budget (8 banks): kvqt 2 + tp 2 + st 2 + ot 2
            with (
                tc.tile_pool(name="xload", bufs=1) as xl_pool,
                tc.tile_pool(name="kv_ps", bufs=2, space="PSUM") as kv_psum,
                tc.tile_pool(name="tp_ps", bufs=1, space="PSUM") as tp_psum,
                tc.tile_pool(name="s_ps", bufs=3, space="PSUM") as s_psum,
                tc.tile_pool(name="ot_ps", bufs=2, space="PSUM") as ot_psum,
                tc.tile_pool(name="pt", bufs=6) as pt_pool,
                tc.tile_pool(name="ep", bufs=2) as ep_pool,
            ):
                reps = int(os.environ.get("K_REPS", "1"))

                def emit_epilogue(j, ot):
                    ots = ep_pool.tile([65, 512], f32, name="ots", tag="ots")
                    nc.vector.tensor_copy(ots[:], ot[:])
                    o_n = ep_pool.tile([128, 4, 65], f32, name="o_n", tag="on")
                    rec = ep_pool.tile([128, 4], f32, name="rec", tag="rc")
                    for u in range(4):
                        tp2 = s_psum.tile([128, 512], f32, name="tp2", tag="st")
                        nc.tensor.transpose(
                            tp2[:, 0:65],
                            ots[:, u * 128 : (u + 1) * 128],
                            ident[0:65, 0:65],
                        )
                        nc.vector.tensor_copy(o_n[:, u, :], tp2[:, 0:65])
                        nc.vector.reciprocal(rec[:, u : u + 1], o_n[:, u, 64:65])
                        nc.vector.tensor_scalar_mul(
                            o_n[:, u, 0:64], o_n[:, u, 0:64], rec[:, u : u + 1]
                        )
                    ov = out[j * 512 : (j + 1) * 512, :].rearrange(
                        "(u p) d -> p u d", p=128
                    )
                    nc.sync.dma_start(ov, o_n[:, :, 0:64])

                for _rep in range(reps):
                    # Front-loaded input staging: one DMA per (d-chunk,
                    # block), issued in usage order — spreads the load
                    # across all DMA queues so the first blocks land fast
                    # and every block has its own fine-grained dependency.
                    xall = xl_pool.tile([128, 6, NSB * 512], bf16, tag="xa")
                    xqall = xl_pool.tile([128, 6, NQB * 512], bf16, tag="xq")
                    for sb in range(NSB):
                        for c in range(6):
                            nc.sync.dma_start(
                                xall[:, c, sb * 512 : (sb + 1) * 512],
                                xt[c * 128 : (c + 1) * 128,
                                   sb * 512 : (sb + 1) * 512],
                            )
                        if sb % 2 == 0:
                            j = sb // 2
                            for c in range(6):
                                nc.sync.dma_start(
                                    xqall[:, c, j * 512 : (j + 1) * 512],
                                    xqt[c * 128 : (c + 1) * 128,
                                        j * 512 : (j + 1) * 512],
                                )
                    for sb in range(NSB):
                        # ---- K/V projection for this s-block ----
                        pass
                        kv = kv_psum.tile([128, 512], f32, name="kv", tag="kvqt")
                        for c in range(6):
                            nc.tensor.matmul(
                                kv[:],
                                wkv[:, c, :],
                                xall[:, c, sb * 512 : (sb + 1) * 512],
                                start=(c == 0),
                                stop=(c == 5),
                            )
                        nc.vector.tensor_copy(
                            KTVT[:, sb * 512 : (sb + 1) * 512], kv[:]
                        )
                        # Vplus chunks for this s-block (V natural layout)
                        for u in range(4):
                            kc = sb * 4 + u
                            vtp = tp_psum.tile([128, 64], bf16, name="vtp", tag="tp")
                            nc.tensor.transpose(
                                vtp[:, :],
                                KTVT[64:128, kc * 128 : (kc + 1) * 128],
                                identb[64:128, 64:128],
                            )
                            nc.vector.tensor_copy(VPr[:, kc, 0:64], vtp[:, :])
                        if sb % 2 == 0:
                            # ---- Q projection for q-block j = sb//2 ----
                            j = sb // 2
                            qt = kv_psum.tile([64, 512], f32, name="qt", tag="kvqt")
                            for c in range(6):
                                nc.tensor.matmul(
                                    qt[:],
                                    wqt[:, c * 64 : (c + 1) * 64],
                                    xqall[:, c, j * 512 : (j + 1) * 512],
                                    start=(c == 0),
                                    stop=(c == 5),
                                )
                            nc.vector.tensor_copy(
                                QT[:, j * 512 : (j + 1) * 512], qt[:]
                            )
                        else:
                            # ---- attention for q-block j = sb//2 ----
                            # Software-pipelined: scores/exp run L chunks
                            # ahead of the PV accumulation so the PE never
                            # waits on the ACT->mask chain.
                            j = sb // 2
                            nkc = 8 * j + 8
                            L = 2
                            ot = ot_psum.tile([65, 512], f32, name="ot", tag="ot")
                            pts = {}
                            # lo[d]: first query column any key of diagonal
                            # chunk d can see — columns below it are fully
                            # masked, so all engines skip them.
                            los = {}
                            for kc in range(nkc + L):
                                if kc < nkc:
                                    d = kc - 8 * j
                                    lo = 0 if d < 2 else 64 * (d - 1)
                                    los[kc] = lo
                                    st = s_psum.tile(
                                        [128, 512], f32, name="st", tag="st"
                                    )
                                    nc.tensor.matmul(
                                        st[:, lo:512],
                                        KTVT[0:64, kc * 128 : (kc + 1) * 128],
                                        QT[:, j * 512 + lo : (j + 1) * 512],
                                        start=True,
                                        stop=True,
                                    )
                                    pt = pt_pool.tile(
                                        [128, 512], bf16, name="pt", tag="pt"
                                    )
                                    nc.scalar.activation(
                                        pt[:, lo:512], st[:, lo:512],
                                        AF.Exp, scale=0.125
                                    )
                                    if d >= 0:
                                        # pt *= (C >= 128d): causal zeroing
                                        nc.vector.scalar_tensor_tensor(
                                            pt[:, lo:512],
                                            cmpt[:, lo:512],
                                            float(128 * d),
                                            pt[:, lo:512],
                                            op0=mybir.AluOpType.is_ge,
                                            op1=mybir.AluOpType.mult,
                                        )
                                    pts[kc] = pt
                                if kc >= L:
                                    lo = los.pop(kc - L)
                                    nc.tensor.matmul(
                                        ot[:, lo:512],
                                        VPr[:, kc - L, :],
                                        pts.pop(kc - L)[:, lo:512],
                                        start=(kc - L == 0),
                                        stop=(kc - L == nkc - 1),
                                    )
                            emit_epilogue(j, ot)

    import concourse.mybir as mybir
    _split_waits(nc, mybir)
    return nc


def _make_runner(nc, n_cores, dev_offset):
    """Compile to a jitted shard_map callable over an explicit device subset."""
    import jax
    import concourse.mybir as mybir
    from concourse import bass2jax
    from jax.experimental.shard_map import shard_map
    from jax.sharding import Mesh, PartitionSpec, NamedSharding

    bass2jax.install_neuronx_cc_hook()

    partition_name = (
        nc.partition_id_tensor.name if nc.partition_id_tensor else None
    )
    in_names, out_names, out_avals, zero_outs = [], [], [], []
    for alloc in nc.m.functions[0].allocations:
        if not isinstance(alloc, mybir.MemoryLocationSet):
            continue
        name = alloc.memorylocations[0].name
        if alloc.kind == "ExternalInput":
            if name != partition_name:
                in_names.append(name)
        elif alloc.kind == "ExternalOutput":
            shape = tuple(alloc.tensor_shape)
            dtype = mybir.dt.np(alloc.dtype)
            out_avals.append(jax.core.ShapedArray(shape, dtype))
            out_names.append(name)
            zero_outs.append(np.zeros(shape, dtype))
    n_params = len(in_names)
    n_outs = len(out_avals)
    all_names = in_names + out_names
    if partition_name is not None:
        all_names.append(partition_name)

    def _body(*args):
        operands = list(args)
        if partition_name is not None:
            operands.append(bass2jax.partition_id_tensor())
        outs = bass2jax._bass_exec_p.bind(
            *operands,
            out_avals=tuple(out_avals),
            in_names=tuple(all_names),
            out_names=tuple(out_names),
            lowering_input_output_aliases=(),
            sim_require_finite=True,
            sim_require_nnan=True,
            nc=nc,
        )
        return tuple(outs)

    devices = jax.devices()[dev_offset : dev_offset + n_cores]
    mesh = Mesh(np.asarray(devices), ("core",))
    in_specs = (PartitionSpec("core"),) * (n_params + n_outs)
    out_specs = (PartitionSpec("core"),) * n_outs
    sharded = jax.jit(
        shard_map(
            _body, mesh=mesh, in_specs=in_specs, out_specs=out_specs, check_rep=False
        ),
        keep_unused=True,
    )
    sh = NamedSharding(mesh, PartitionSpec("core"))

    def prepare(in_maps):
        per_core = [[np.asarray(m[n]) for n in in_names] for m in in_maps]
        concat_in = [
            np.concatenate([per_core[c][i] for c in range(n_cores)], axis=0)
            for i in range(n_params)
        ]
        concat_zeros = [
            np.zeros((n_cores * z.shape[0], *z.shape[1:]), z.dtype)
            for z in zero_outs
        ]
        return [jax.device_put(a, sh) for a in concat_in + concat_zeros]

    def run(in_maps):
        return sharded(*prepare(in_maps))

    run.sharded = sharded
    run.prepare = prepare
    run.out_names = out_names
    run.out_avals = out_avals
    run.n_cores = n_cores
    return run


def _bf16():
    import ml_dtypes

    return ml_dtypes.bfloat16


def _make_cmp(h):
    """Causal comparison base for interleave offset h.

    C[p, i] = 2i + h - p.  Key 128*(8j+d)+p is visible to local query
    1024j+2i+h iff C[p, i] >= 128d (j-independent), applied on-device as
    pt *= (C >= 128d) for the 8 diagonal chunks d."""
    p = np.arange(128)[:, None]
    i = np.arange(512)[None, :]
    return (2 * i + h - p).astype(np.float32)


def _get_runner():
    if "runner" not in _cache:
        nc = _build_program()
        _cache["runner"] = _make_runner(nc, NCORE, 0)
    return _cache["runner"]


def _core_maps(x, WQ, WK, WV):
    bf = _bf16()
    cmps = [_make_cmp(0), _make_cmp(1)]
    maps = []
    for c in range(NCORE):
        b, h = c // 2, c % 2
        maps.append(
            {
                "xt": np.ascontiguousarray(x[b].T).astype(bf),
                "xqt": np.ascontiguousarray(x[b][h::2].T).astype(bf),
                "wq": WQ.astype(bf),
                "wk": WK.astype(bf),
                "wv": WV.astype(bf),
                "cmp": cmps[h],
            }
        )
    return maps


def kernel(x, WQ, WK, WV):
    run = _get_runner()
    res = run(_core_maps(x, WQ, WK, WV))
    halves = np.asarray(res[0]).reshape(NCORE, SQ, DK)
    out = np.empty((B, S, DK), np.float32)
    for c in range(NCORE):
        b, h = c // 2, c % 2
        out[b, h::2] = halves[c]
    return out


if __name__ == "__main__":
    rng = np.random.default_rng(0)
    x = rng.standard_normal((B, S, DM), dtype=np.float32)
    sc = 1.0 / np.sqrt(DM)
    WQ = rng.standard_normal((DM, DK), dtype=np.float32) * sc
    WK = rng.standard_normal((DM, DK), dtype=np.float32) * sc
    WV = rng.standard_normal((DM, DK), dtype=np.float32) * sc
    got = kernel(x, WQ, WK, WV)
    # numpy reference
    Q = x @ WQ
    K = x @ WK
    V = x @ WV
    sref = np.einsum("bqd,bkd->bqk", Q, K) / 8.0
    mask = np.tril(np.ones((S, S), bool))
    sref = np.where(mask, sref, -np.inf)
    sref = sref - sref.max(-1, keepdims=True)
    p = np.exp(sref)
    p /= p.sum(-1, keepdims=True)
    ref = np.einsum("bqk,bkv->bqv", p, V)
    err = np.abs(got - ref).max() / np.abs(ref).max()
    print("rel err:", err)


# revision 14
# speedup vs baseline: 6.6058x; 5.4305x over previous
"""Causal single-head attention on 8 trn2 NeuronCores — single SPMD program.

Problem: x[4,4096,768], WQ/WK/WV[768,64] -> out[4,4096,64]
  Q=x@WQ K=x@WK V=x@WV; causal softmax(QK^T/8)@V per batch.

Sharding: core c handles batch b=c//2 with query-interleave h=c%2 (its
queries are global rows h, h+2, h+4, ... of the batch).  Interleaving
makes the causal structure IDENTICAL on all 8 cores: local q-block j
(512 queries spanning global rows 1024j+h .. 1024j+1022+h) always sees
k-chunks 0..8j+7.  The only h-dependence is a +-1 shift of the diagonal
mask boundary, which is supplied as a tiny per-core 0/1 mask INPUT and
applied with one vector multiply after exp — so one program serves all
cores and the whole forward pass is a single 8-device dispatch.

Host staging (outside the kernel): x is pre-transposed per core
(xt = x[b].T for K/V, xqt = x[b][h::2].T for Q), which removes all PE
transposes of x from the device kernel.

Kernel per core (all fp32):
  For each s-block sb (512 keys): DMA xT tile, project [K^T|V^T] with
  fused [WK|WV] stationary; build Vplus=[V|1] chunks via PE transpose.
  On even sb: project Q^T for q-block j=sb//2.  After odd sb: flash
  attention for q-block j=sb//2 with scores TRANSPOSED (keys on
  partitions): scoresT[k,q] = matmul(KT chunk, QT block); exp on ACT
  (scale=1/8, no max subtraction — scores ~ N(0,1), safe in fp32);
  causal zeroing on the 8 diagonal chunks via mask multiply;
  OT[65,q] += matmul(Vplus[128,65], PT) — row 64 = softmax denominator.
  Epilogue: PE-transpose OT -> O natural, reciprocal * scale, DMA out.
"""
import sys
import os

sys.path.insert(0, "/opt/trn_rl_repo")

import numpy as np

B, S, DM, DK = 4, 4096, 768, 64
NCORE = 8
SQ = S // 2          # queries per core (interleaved)
NSB = S // 512       # 8 s-blocks of 512 keys
NQB = SQ // 512      # 4 local q-blocks of 512 queries

_cache = {}


def _split_waits(nc, mybir, maxw=1):
    """Walrus here accepts only 1 sem-wait per instruction; move excess
    waits onto preceding same-engine no-ops."""
    cnt = 0
    for bb in nc.m.functions[0].blocks:
        new_insts = []
        for inst in bb.instructions:
            si = inst.sync_info
            if si is not None and si.on_wait and len(si.on_wait) > maxw:
                waits = list(si.on_wait)
                si.on_wait = waits[:maxw]
                extra = waits[maxw:]
                for i in range(0, len(extra), maxw):
                    cnt += 1
                    nop = mybir.InstNoOp(name=f"waitsplit-{cnt}", ins=[], outs=[])
                    nop.engine = inst.engine
                    nop.sync_info = mybir.SyncInfo(
                        on_wait=extra[i : i + maxw], on_update=[]
                    )
                    new_insts.append(nop)
            new_insts.append(inst)
        bb.instructions[:] = new_insts


def _build_program(reps=1):
    import concourse.bass as bass
    import concourse.mybir as mybir
    from concourse.tile import TileContext
    from concourse.masks import make_identity

    f32 = mybir.dt.float32
    bf16 = mybir.dt.bfloat16
    AF = mybir.ActivationFunctionType

    nc = bass.Bass()
    xt = nc.declare_dram_parameter("xt", [DM, S], bf16, isOutput=False)
    xqt = nc.declare_dram_parameter("xqt", [DM, SQ], bf16, isOutput=False)
    wq = nc.declare_dram_parameter("wq", [DM, DK], bf16, isOutput=False)
    wk = nc.declare_dram_parameter("wk", [DM, DK], bf16, isOutput=False)
    wv = nc.declare_dram_parameter("wv", [DM, DK], bf16, isOutput=False)
    cmp_ = nc.declare_dram_parameter("cmp", [128, 512], f32, isOutput=False)
    out = nc.declare_dram_parameter("out", [SQ, DK], f32, isOutput=True)

    with TileContext(nc) as tc:
        with (
            tc.tile_pool(name="consts", bufs=1) as cpool,
            tc.tile_pool(name="big", bufs=1) as big,
        ):
            ident = cpool.tile([128, 128], f32)
            make_identity(nc, ident[:])
            identb = cpool.tile([128, 128], bf16)
            nc.vector.tensor_copy(identb[:], ident[:])
            # [WK|WV] stationary chunks: cols 0:64 = WK, 64:128 = WV.
            # One strided DMA per weight tensor (dest viewed [128, 6, .]).
            wkv = cpool.tile([128, 6, 128], bf16)
            wqt = cpool.tile([128, 6 * 64], bf16)
            nc.sync.dma_start(
                wkv[:, :, 0:64], wk[:].rearrange("(c p) d -> p c d", p=128)
            )
            nc.sync.dma_start(
                wkv[:, :, 64:128], wv[:].rearrange("(c p) d -> p c d", p=128)
            )
            nc.sync.dma_start(
                wqt[:].rearrange("p (c d) -> p c d", d=64),
                wq[:].rearrange("(c p) d -> p c d", p=128),
            )
            cmpt = cpool.tile([128, 512], f32)
            nc.sync.dma_start(cmpt[:], cmp_[:])

            KTVT = big.tile([128, S], bf16)  # rows 0:64 = K^T, 64:128 = V^T
            QT = big.tile([64, SQ], bf16)  # own queries, transposed
            VP = big.tile([128, 32 * 65], bf16)  # [V|1] per k-chunk
            VPr = VP[:].rearrange("p (c u) -> p c u", u=65)
            nc.gpsimd.memset(VP[:], 1.0)  # ones column; V part overwritten

            # PSUM budget (8 banks): kvqt 2 + tp 2 + st 2 + ot 2
            with (
                tc.tile_pool(name="xload", bufs=1) as xl_pool,
                tc.tile_pool(name="kv_ps", bufs=2, space="PSUM") as kv_psum,
                tc.tile_pool(name="tp_ps", bufs=1, space="PSUM") as tp_psum,
                tc.tile_pool(name="s_ps", bufs=3, space="PSUM") as s_psum,
                tc.tile_pool(name="ot_ps", bufs=2, space="PSUM") as ot_psum,
                tc.tile_pool(name="pt", bufs=6) as pt_pool,
                tc.tile_pool(name="ep", bufs=2) as ep_pool,
            ):


                def emit_epilogue(j, ot):
                    ots = ep_pool.tile([65, 512], f32, name="ots", tag="ots")
                    nc.vector.tensor_copy(ots[:], ot[:])
                    o_n = ep_pool.tile([128, 4, 65], f32, name="o_n", tag="on")
                    rec = ep_pool.tile([128, 4], f32, name="rec", tag="rc")
                    for u in range(4):
                        tp2 = s_psum.tile([128, 512], f32, name="tp2", tag="st")
                        nc.tensor.transpose(
                            tp2[:, 0:65],
                            ots[:, u * 128 : (u + 1) * 128],
                            ident[0:65, 0:65],
                        )
                        nc.vector.tensor_copy(o_n[:, u, :], tp2[:, 0:65])
                        nc.vector.reciprocal(rec[:, u : u + 1], o_n[:, u, 64:65])
                        nc.vector.tensor_scalar_mul(
                            o_n[:, u, 0:64], o_n[:, u, 0:64], rec[:, u : u + 1]
                        )
                    ov = out[j * 512 : (j + 1) * 512, :].rearrange(
                        "(u p) d -> p u d", p=128
                    )
                    nc.sync.dma_start(ov, o_n[:, :, 0:64])

                for _rep in range(reps):
                    # Front-loaded input staging: one DMA per (d-chunk,
                    # block), issued in usage order — spreads the load
                    # across all DMA queues so the first blocks land fast
                    # and every block has its own fine-grained dependency.
                    xall = xl_pool.tile([128, 6, NSB * 512], bf16, tag="xa")
                    xqall = xl_pool.tile([128, 6, NQB * 512], bf16, tag="xq")
                    for sb in range(NSB):
                        for c in range(6):
                            nc.sync.dma_start(
                                xall[:, c, sb * 512 : (sb + 1) * 512],
                                xt[c * 128 : (c + 1) * 128,
                                   sb * 512 : (sb + 1) * 512],
                            )
                        if sb % 2 == 0:
                            j = sb // 2
                            for c in range(6):
                                nc.sync.dma_start(
                                    xqall[:, c, j * 512 : (j + 1) * 512],
                                    xqt[c * 128 : (c + 1) * 128,
                                        j * 512 : (j + 1) * 512],
                                )
                    for sb in range(NSB):
                        # ---- K/V projection for this s-block ----
                        pass
                        kv = kv_psum.tile([128, 512], f32, name="kv", tag="kvqt")
                        for c in range(6):
                            nc.tensor.matmul(
                                kv[:],
                                wkv[:, c, :],
                                xall[:, c, sb * 512 : (sb + 1) * 512],
                                start=(c == 0),
                                stop=(c == 5),
                            )
                        nc.vector.tensor_copy(
                            KTVT[:, sb * 512 : (sb + 1) * 512], kv[:]
                        )
                        # Vplus chunks for this s-block (V natural layout)
                        for u in range(4):
                            kc = sb * 4 + u
                            vtp = tp_psum.tile([128, 64], bf16, name="vtp", tag="tp")
                            nc.tensor.transpose(
                                vtp[:, :],
                                KTVT[64:128, kc * 128 : (kc + 1) * 128],
                                identb[64:128, 64:128],
                            )
                            nc.vector.tensor_copy(VPr[:, kc, 0:64], vtp[:, :])
                        if sb % 2 == 0:
                            # ---- Q projection for q-block j = sb//2 ----
                            j = sb // 2
                            qt = kv_psum.tile([64, 512], f32, name="qt", tag="kvqt")
                            for c in range(6):
                                nc.tensor.matmul(
                                    qt[:],
                                    wqt[:, c * 64 : (c + 1) * 64],
                                    xqall[:, c, j * 512 : (j + 1) * 512],
                                    start=(c == 0),
                                    stop=(c == 5),
                                )
                            nc.vector.tensor_copy(
                                QT[:, j * 512 : (j + 1) * 512], qt[:]
                            )
                        else:
                            # ---- attention for q-block j = sb//2 ----
                            # Software-pipelined: scores/exp run L chunks
                            # ahead of the PV accumulation so the PE never
                            # waits on the ACT->mask chain.
                            j = sb // 2
                            nkc = 8 * j + 8
                            L = 2
                            ot = ot_psum.tile([65, 512], f32, name="ot", tag="ot")
                            pts = {}
                            # lo[d]: first query column any key of diagonal
                            # chunk d can see — columns below it are fully
                            # masked, so all engines skip them.
                            los = {}
                            for kc in range(nkc + L):
                                if kc < nkc:
                                    d = kc - 8 * j
                                    lo = 0 if d < 2 else 64 * (d - 1)
                                    los[kc] = lo
                                    st = s_psum.tile(
                                        [128, 512], f32, name="st", tag="st"
                                    )
                                    nc.tensor.matmul(
                                        st[:, lo:512],
                                        KTVT[0:64, kc * 128 : (kc + 1) * 128],
                                        QT[:, j * 512 + lo : (j + 1) * 512],
                                        start=True,
                                        stop=True,
                                    )
                                    pt = pt_pool.tile(
                                        [128, 512], bf16, name="pt", tag="pt"
                                    )
                                    nc.scalar.activation(
                                        pt[:, lo:512], st[:, lo:512],
                                        AF.Exp, scale=0.125
                                    )
                                    if d >= 0:
                                        # pt *= (C >= 128d): causal zeroing
                                        nc.vector.scalar_tensor_tensor(
                                            pt[:, lo:512],
                                            cmpt[:, lo:512],
                                            float(128 * d),
                                            pt[:, lo:512],
                                            op0=mybir.AluOpType.is_ge,
                                            op1=mybir.AluOpType.mult,
                                        )
                                    pts[kc] = pt
                                if kc >= L:
                                    lo = los.pop(kc - L)
                                    nc.tensor.matmul(
                                        ot[:, lo:512],
                                        VPr[:, kc - L, :],
                                        pts.pop(kc - L)[:, lo:512],
                                        start=(kc - L == 0),
                                        stop=(kc - L == nkc - 1),
                                    )
                            emit_epilogue(j, ot)

    import concourse.mybir as mybir
    _split_waits(nc, mybir)
    return nc


def _make_runner(nc, n_cores, dev_offset):
    """Compile to a jitted shard_map callable over an explicit device subset."""
    import jax
    import concourse.mybir as mybir
    from concourse import bass2jax
    from jax.experimental.shard_map import shard_map
    from jax.sharding import Mesh, PartitionSpec, NamedSharding

    bass2jax.install_neuronx_cc_hook()

    partition_name = (
        nc.partition_id_tensor.name if nc.partition_id_tensor else None
    )
    in_names, out_names, out_avals, zero_outs = [], [], [], []
    for alloc in nc.m.functions[0].allocations:
        if not isinstance(alloc, mybir.MemoryLocationSet):
            continue
        name = alloc.memorylocations[0].name
        if alloc.kind == "ExternalInput":
            if name != partition_name:
                in_names.append(name)
        elif alloc.kind == "ExternalOutput":
            shape = tuple(alloc.tensor_shape)
            dtype = mybir.dt.np(alloc.dtype)
            out_avals.append(jax.core.ShapedArray(shape, dtype))
            out_names.append(name)
            zero_outs.append(np.zeros(shape, dtype))
    n_params = len(in_names)
    n_outs = len(out_avals)
    all_names = in_names + out_names
    if partition_name is not None:
        all_names.append(partition_name)

    def _body(*args):
        operands = list(args)
        if partition_name is not None:
            operands.append(bass2jax.partition_id_tensor())
        outs = bass2jax._bass_exec_p.bind(
            *operands,
            out_avals=tuple(out_avals),
            in_names=tuple(all_names),
            out_names=tuple(out_names),
            lowering_input_output_aliases=(),
            sim_require_finite=True,
            sim_require_nnan=True,
            nc=nc,
        )
        return tuple(outs)

    devices = jax.devices()[dev_offset : dev_offset + n_cores]
    mesh = Mesh(np.asarray(devices), ("core",))
    in_specs = (PartitionSpec("core"),) * (n_params + n_outs)
    out_specs = (PartitionSpec("core"),) * n_outs
    sharded = jax.jit(
        shard_map(
            _body, mesh=mesh, in_specs=in_specs, out_specs=out_specs, check_rep=False
        ),
        keep_unused=True,
    )
    sh = NamedSharding(mesh, PartitionSpec("core"))

    def prepare(in_maps):
        per_core = [[np.asarray(m[n]) for n in in_names] for m in in_maps]
        concat_in = [
            np.concatenate([per_core[c][i] for c in range(n_cores)], axis=0)
            for i in range(n_params)
        ]
        concat_zeros = [
            np.zeros((n_cores * z.shape[0], *z.shape[1:]), z.dtype)
            for z in zero_outs
        ]
        return [jax.device_put(a, sh) for a in concat_in + concat_zeros]

    def run(in_maps):
        return sharded(*prepare(in_maps))

    run.sharded = sharded
    run.prepare = prepare
    run.out_names = out_names
    run.out_avals = out_avals
    run.n_cores = n_cores
    return run


def _bf16():
    import ml_dtypes

    return ml_dtypes.bfloat16


def _make_cmp(h):
    """Causal comparison base for interleave offset h.

    C[p, i] = 2i + h - p.  Key 128*(8j+d)+p is visible to local query
    1024j+2i+h iff C[p, i] >= 128d (j-independent), applied on-device as
    pt *= (C >= 128d) for the 8 diagonal chunks d."""
    p = np.arange(128)[:, None]
    i = np.arange(512)[None, :]
    return (2 * i + h - p).astype(np.float32)


def _get_runner(reps=1):
    key = ("runner", reps)
    if key not in _cache:
        nc = _build_program(reps)
        _cache[key] = _make_runner(nc, NCORE, 0)
    return _cache[key]


def _core_maps(x, WQ, WK, WV):
    bf = _bf16()
    cmps = [_make_cmp(0), _make_cmp(1)]
    maps = []
    for c in range(NCORE):
        b, h = c // 2, c % 2
        maps.append(
            {
                "xt": np.ascontiguousarray(x[b].T).astype(bf),
                "xqt": np.ascontiguousarray(x[b][h::2].T).astype(bf),
                "wq": WQ.astype(bf),
                "wk": WK.astype(bf),
                "wv": WV.astype(bf),
                "cmp": cmps[h],
            }
        )
    return maps


def kernel(x, WQ, WK, WV):
    run = _get_runner()
    res = run(_core_maps(x, WQ, WK, WV))
    halves = np.asarray(res[0]).reshape(NCORE, SQ, DK)
    out = np.empty((B, S, DK), np.float32)
    for c in range(NCORE):
        b, h = c // 2, c % 2
        out[b, h::2] = halves[c]
    return out


if __name__ == "__main__":
    rng = np.random.default_rng(0)
    x = rng.standard_normal((B, S, DM), dtype=np.float32)
    sc = 1.0 / np.sqrt(DM)
    WQ = rng.standard_normal((DM, DK), dtype=np.float32) * sc
    WK = rng.standard_normal((DM, DK), dtype=np.float32) * sc
    WV = rng.standard_normal((DM, DK), dtype=np.float32) * sc
    got = kernel(x, WQ, WK, WV)
    # numpy reference
    Q = x @ WQ
    K = x @ WK
    V = x @ WV
    sref = np.einsum("bqd,bkd->bqk", Q, K) / 8.0
    mask = np.tril(np.ones((S, S), bool))
    sref = np.where(mask, sref, -np.inf)
    sref = sref - sref.max(-1, keepdims=True)
    p = np.exp(sref)
    p /= p.sum(-1, keepdims=True)
    ref = np.einsum("bqk,bkv->bqv", p, V)
    err = np.abs(got - ref).max() / np.abs(ref).max()
    print("rel err:", err)


# revision 15
# speedup vs baseline: 17.2974x; 2.6185x over previous
"""Causal single-head attention on 8 trn2 NeuronCores — single SPMD program.

Problem: x[4,4096,768], WQ/WK/WV[768,64] -> out[4,4096,64]
  Q=x@WQ K=x@WK V=x@WV; causal softmax(QK^T/8)@V per batch.

Sharding: core c handles batch b=c//2 with query-interleave h=c%2 (its
queries are global rows h, h+2, h+4, ... of the batch).  Interleaving
makes the causal structure IDENTICAL on all 8 cores: local q-block j
(512 queries spanning global rows 1024j+h .. 1024j+1022+h) always sees
k-chunks 0..8j+7.  The only h-dependence is a +-1 shift of the diagonal
mask boundary, which is supplied as a tiny per-core 0/1 mask INPUT and
applied with one vector multiply after exp — so one program serves all
cores and the whole forward pass is a single 8-device dispatch.

Host staging (outside the kernel): x is pre-transposed per core
(xt = x[b].T for K/V, xqt = x[b][h::2].T for Q), which removes all PE
transposes of x from the device kernel.

Kernel per core (all fp32):
  For each s-block sb (512 keys): DMA xT tile, project [K^T|V^T] with
  fused [WK|WV] stationary; build Vplus=[V|1] chunks via PE transpose.
  On even sb: project Q^T for q-block j=sb//2.  After odd sb: flash
  attention for q-block j=sb//2 with scores TRANSPOSED (keys on
  partitions): scoresT[k,q] = matmul(KT chunk, QT block); exp on ACT
  (scale=1/8, no max subtraction — scores ~ N(0,1), safe in fp32);
  causal zeroing on the 8 diagonal chunks via mask multiply;
  OT[65,q] += matmul(Vplus[128,65], PT) — row 64 = softmax denominator.
  Epilogue: PE-transpose OT -> O natural, reciprocal * scale, DMA out.
"""
import sys
import os

sys.path.insert(0, "/opt/trn_rl_repo")

import numpy as np

B, S, DM, DK = 4, 4096, 768, 64
NCORE = 8
SQ = S // 2          # queries per core (interleaved)
NSB = S // 512       # 8 s-blocks of 512 keys
NQB = SQ // 512      # 4 local q-blocks of 512 queries

_cache = {}


def _split_waits(nc, mybir, maxw=1):
    """Walrus here accepts only 1 sem-wait per instruction; move excess
    waits onto preceding same-engine no-ops."""
    cnt = 0
    for bb in nc.m.functions[0].blocks:
        new_insts = []
        for inst in bb.instructions:
            si = inst.sync_info
            if si is not None and si.on_wait and len(si.on_wait) > maxw:
                waits = list(si.on_wait)
                si.on_wait = waits[:maxw]
                extra = waits[maxw:]
                for i in range(0, len(extra), maxw):
                    cnt += 1
                    nop = mybir.InstNoOp(name=f"waitsplit-{cnt}", ins=[], outs=[])
                    nop.engine = inst.engine
                    nop.sync_info = mybir.SyncInfo(
                        on_wait=extra[i : i + maxw], on_update=[]
                    )
                    new_insts.append(nop)
            new_insts.append(inst)
        bb.instructions[:] = new_insts


def _build_program(reps=1):
    import concourse.bass as bass
    import concourse.mybir as mybir
    from concourse.tile import TileContext
    from concourse.masks import make_identity

    f32 = mybir.dt.float32
    bf16 = mybir.dt.bfloat16
    AF = mybir.ActivationFunctionType

    nc = bass.Bass()
    xt = nc.declare_dram_parameter("xt", [DM, S], bf16, isOutput=False)
    xqt = nc.declare_dram_parameter("xqt", [DM, SQ], bf16, isOutput=False)
    wq = nc.declare_dram_parameter("wq", [DM, DK], bf16, isOutput=False)
    wk = nc.declare_dram_parameter("wk", [DM, DK], bf16, isOutput=False)
    wv = nc.declare_dram_parameter("wv", [DM, DK], bf16, isOutput=False)
    cmp_ = nc.declare_dram_parameter("cmp", [128, 512], f32, isOutput=False)
    out = nc.declare_dram_parameter("out", [SQ, DK], f32, isOutput=True)

    with TileContext(nc) as tc:
        with (
            tc.tile_pool(name="consts", bufs=1) as cpool,
            tc.tile_pool(name="big", bufs=1) as big,
        ):
            ident = cpool.tile([128, 128], f32)
            make_identity(nc, ident[:])
            identb = cpool.tile([128, 128], bf16)
            nc.vector.tensor_copy(identb[:], ident[:])
            # [WK|WV] stationary chunks: cols 0:64 = WK, 64:128 = WV.
            # One strided DMA per weight tensor (dest viewed [128, 6, .]).
            wkv = cpool.tile([128, 6, 128], bf16)
            wqt = cpool.tile([128, 6 * 64], bf16)
            nc.sync.dma_start(
                wkv[:, :, 0:64], wk[:].rearrange("(c p) d -> p c d", p=128)
            )
            nc.sync.dma_start(
                wkv[:, :, 64:128], wv[:].rearrange("(c p) d -> p c d", p=128)
            )
            nc.sync.dma_start(
                wqt[:].rearrange("p (c d) -> p c d", d=64),
                wq[:].rearrange("(c p) d -> p c d", p=128),
            )
            cmpt = cpool.tile([128, 512], f32)
            nc.sync.dma_start(cmpt[:], cmp_[:])

            KTVT = big.tile([128, S], bf16)  # rows 0:64 = K^T, 64:128 = V^T
            QT = big.tile([64, SQ], bf16)  # own queries, transposed
            VP = big.tile([128, 32 * 65], bf16)  # [V|1] per k-chunk
            VPr = VP[:].rearrange("p (c u) -> p c u", u=65)
            nc.gpsimd.memset(VP[:], 1.0)  # ones column; V part overwritten

            # PSUM budget (8 banks): kvqt 2 + tp 2 + st 2 + ot 2
            with (
                tc.tile_pool(name="xload", bufs=2) as xl_pool,
                tc.tile_pool(name="kv_ps", bufs=2, space="PSUM") as kv_psum,
                tc.tile_pool(name="tp_ps", bufs=1, space="PSUM") as tp_psum,
                tc.tile_pool(name="s_ps", bufs=3, space="PSUM") as s_psum,
                tc.tile_pool(name="ot_ps", bufs=2, space="PSUM") as ot_psum,
                tc.tile_pool(name="pt", bufs=6) as pt_pool,
                tc.tile_pool(name="ep", bufs=2) as ep_pool,
            ):


                def emit_epilogue(j, ot):
                    ots = ep_pool.tile([65, 512], f32, name="ots", tag="ots")
                    nc.vector.tensor_copy(ots[:], ot[:])
                    o_n = ep_pool.tile([128, 4, 65], f32, name="o_n", tag="on")
                    rec = ep_pool.tile([128, 4], f32, name="rec", tag="rc")
                    for u in range(4):
                        tp2 = s_psum.tile([128, 512], f32, name="tp2", tag="st")
                        nc.tensor.transpose(
                            tp2[:, 0:65],
                            ots[:, u * 128 : (u + 1) * 128],
                            ident[0:65, 0:65],
                        )
                        nc.vector.tensor_copy(o_n[:, u, :], tp2[:, 0:65])
                        nc.vector.reciprocal(rec[:, u : u + 1], o_n[:, u, 64:65])
                        nc.vector.tensor_scalar_mul(
                            o_n[:, u, 0:64], o_n[:, u, 0:64], rec[:, u : u + 1]
                        )
                    ov = out[j * 512 : (j + 1) * 512, :].rearrange(
                        "(u p) d -> p u d", p=128
                    )
                    nc.sync.dma_start(ov, o_n[:, :, 0:64])

                for _rep in range(reps):
                    # Input staging in double-buffered 2-block tiles: one
                    # DMA per (d-chunk, block) spreads load across DMA
                    # queues; pair granularity keeps deps fine-grained and
                    # lets the next rep's DMA overlap this rep's tail.
                    xprs, xqprs = [], []
                    for pr in range(NSB // 2):
                        xpr = xl_pool.tile([128, 6, 1024], bf16, tag="xa")
                        xprs.append(xpr)
                        for half in range(2):
                            sb = pr * 2 + half
                            for c in range(6):
                                nc.sync.dma_start(
                                    xpr[:, c, half * 512 : (half + 1) * 512],
                                    xt[c * 128 : (c + 1) * 128,
                                       sb * 512 : (sb + 1) * 512],
                                )
                        if pr % 2 == 0:
                            xqpr = xl_pool.tile([128, 6, 1024], bf16, tag="xq")
                            xqprs.append(xqpr)
                            for half in range(2):
                                j = pr + half
                                for c in range(6):
                                    nc.sync.dma_start(
                                        xqpr[:, c, half * 512 : (half + 1) * 512],
                                        xqt[c * 128 : (c + 1) * 128,
                                            j * 512 : (j + 1) * 512],
                                    )
                    for sb in range(NSB):
                        # ---- K/V projection for this s-block ----
                        pass
                        kv = kv_psum.tile([128, 512], f32, name="kv", tag="kvqt")
                        for c in range(6):
                            nc.tensor.matmul(
                                kv[:],
                                wkv[:, c, :],
                                xprs[sb // 2][
                                    :, c, (sb % 2) * 512 : (sb % 2 + 1) * 512
                                ],
                                start=(c == 0),
                                stop=(c == 5),
                            )
                        nc.vector.tensor_copy(
                            KTVT[:, sb * 512 : (sb + 1) * 512], kv[:]
                        )
                        # Vplus chunks for this s-block (V natural layout)
                        for u in range(4):
                            kc = sb * 4 + u
                            vtp = tp_psum.tile([128, 64], bf16, name="vtp", tag="tp")
                            nc.tensor.transpose(
                                vtp[:, :],
                                KTVT[64:128, kc * 128 : (kc + 1) * 128],
                                identb[64:128, 64:128],
                            )
                            nc.vector.tensor_copy(VPr[:, kc, 0:64], vtp[:, :])
                        if sb % 2 == 0:
                            # ---- Q projection for q-block j = sb//2 ----
                            j = sb // 2
                            qt = kv_psum.tile([64, 512], f32, name="qt", tag="kvqt")
                            for c in range(6):
                                nc.tensor.matmul(
                                    qt[:],
                                    wqt[:, c * 64 : (c + 1) * 64],
                                    xqprs[j // 2][
                                        :, c, (j % 2) * 512 : (j % 2 + 1) * 512
                                    ],
                                    start=(c == 0),
                                    stop=(c == 5),
                                )
                            nc.vector.tensor_copy(
                                QT[:, j * 512 : (j + 1) * 512], qt[:]
                            )
                        else:
                            # ---- attention for q-block j = sb//2 ----
                            # Software-pipelined: scores/exp run L chunks
                            # ahead of the PV accumulation so the PE never
                            # waits on the ACT->mask chain.
                            j = sb // 2
                            nkc = 8 * j + 8
                            L = 2
                            ot = ot_psum.tile([65, 512], f32, name="ot", tag="ot")
                            pts = {}
                            # lo[d]: first query column any key of diagonal
                            # chunk d can see — columns below it are fully
                            # masked, so all engines skip them.
                            los = {}
                            for kc in range(nkc + L):
                                if kc < nkc:
                                    d = kc - 8 * j
                                    lo = 0 if d < 2 else 64 * (d - 1)
                                    los[kc] = lo
                                    st = s_psum.tile(
                                        [128, 512], f32, name="st", tag="st"
                                    )
                                    nc.tensor.matmul(
                                        st[:, lo:512],
                                        KTVT[0:64, kc * 128 : (kc + 1) * 128],
                                        QT[:, j * 512 + lo : (j + 1) * 512],
                                        start=True,
                                        stop=True,
                                    )
                                    pt = pt_pool.tile(
                                        [128, 512], bf16, name="pt", tag="pt"
                                    )
                                    nc.scalar.activation(
                                        pt[:, lo:512], st[:, lo:512],
                                        AF.Exp, scale=0.125
                                    )
                                    if d >= 0:
                                        # pt *= (C >= 128d): causal zeroing
                                        nc.vector.scalar_tensor_tensor(
                                            pt[:, lo:512],
                                            cmpt[:, lo:512],
                                            float(128 * d),
                                            pt[:, lo:512],
                                            op0=mybir.AluOpType.is_ge,
                                            op1=mybir.AluOpType.mult,
                                        )
                                    pts[kc] = pt
                                if kc >= L:
                                    lo = los.pop(kc - L)
                                    nc.tensor.matmul(
                                        ot[:, lo:512],
                                        VPr[:, kc - L, :],
                                        pts.pop(kc - L)[:, lo:512],
                                        start=(kc - L == 0),
                                        stop=(kc - L == nkc - 1),
                                    )
                            emit_epilogue(j, ot)

    import concourse.mybir as mybir
    _split_waits(nc, mybir)
    return nc


def _make_runner(nc, n_cores, dev_offset):
    """Compile to a jitted shard_map callable over an explicit device subset."""
    import jax
    import concourse.mybir as mybir
    from concourse import bass2jax
    from jax.experimental.shard_map import shard_map
    from jax.sharding import Mesh, PartitionSpec, NamedSharding

    bass2jax.install_neuronx_cc_hook()

    partition_name = (
        nc.partition_id_tensor.name if nc.partition_id_tensor else None
    )
    in_names, out_names, out_avals, zero_outs = [], [], [], []
    for alloc in nc.m.functions[0].allocations:
        if not isinstance(alloc, mybir.MemoryLocationSet):
            continue
        name = alloc.memorylocations[0].name
        if alloc.kind == "ExternalInput":
            if name != partition_name:
                in_names.append(name)
        elif alloc.kind == "ExternalOutput":
            shape = tuple(alloc.tensor_shape)
            dtype = mybir.dt.np(alloc.dtype)
            out_avals.append(jax.core.ShapedArray(shape, dtype))
            out_names.append(name)
            zero_outs.append(np.zeros(shape, dtype))
    n_params = len(in_names)
    n_outs = len(out_avals)
    all_names = in_names + out_names
    if partition_name is not None:
        all_names.append(partition_name)

    def _body(*args):
        operands = list(args)
        if partition_name is not None:
            operands.append(bass2jax.partition_id_tensor())
        outs = bass2jax._bass_exec_p.bind(
            *operands,
            out_avals=tuple(out_avals),
            in_names=tuple(all_names),
            out_names=tuple(out_names),
            lowering_input_output_aliases=(),
            sim_require_finite=True,
            sim_require_nnan=True,
            nc=nc,
        )
        return tuple(outs)

    devices = jax.devices()[dev_offset : dev_offset + n_cores]
    mesh = Mesh(np.asarray(devices), ("core",))
    in_specs = (PartitionSpec("core"),) * (n_params + n_outs)
    out_specs = (PartitionSpec("core"),) * n_outs
    sharded = jax.jit(
        shard_map(
            _body, mesh=mesh, in_specs=in_specs, out_specs=out_specs, check_rep=False
        ),
        keep_unused=True,
    )
    sh = NamedSharding(mesh, PartitionSpec("core"))

    def prepare(in_maps):
        per_core = [[np.asarray(m[n]) for n in in_names] for m in in_maps]
        concat_in = [
            np.concatenate([per_core[c][i] for c in range(n_cores)], axis=0)
            for i in range(n_params)
        ]
        concat_zeros = [
            np.zeros((n_cores * z.shape[0], *z.shape[1:]), z.dtype)
            for z in zero_outs
        ]
        return [jax.device_put(a, sh) for a in concat_in + concat_zeros]

    def run(in_maps):
        return sharded(*prepare(in_maps))

    run.sharded = sharded
    run.prepare = prepare
    run.out_names = out_names
    run.out_avals = out_avals
    run.n_cores = n_cores
    return run


def _bf16():
    import ml_dtypes

    return ml_dtypes.bfloat16


def _make_cmp(h):
    """Causal comparison base for interleave offset h.

    C[p, i] = 2i + h - p.  Key 128*(8j+d)+p is visible to local query
    1024j+2i+h iff C[p, i] >= 128d (j-independent), applied on-device as
    pt *= (C >= 128d) for the 8 diagonal chunks d."""
    p = np.arange(128)[:, None]
    i = np.arange(512)[None, :]
    return (2 * i + h - p).astype(np.float32)


def _get_runner(reps=1):
    key = ("runner", reps)
    if key not in _cache:
        nc = _build_program(reps)
        _cache[key] = _make_runner(nc, NCORE, 0)
    return _cache[key]


def _core_maps(x, WQ, WK, WV):
    bf = _bf16()
    cmps = [_make_cmp(0), _make_cmp(1)]
    maps = []
    for c in range(NCORE):
        b, h = c // 2, c % 2
        maps.append(
            {
                "xt": np.ascontiguousarray(x[b].T).astype(bf),
                "xqt": np.ascontiguousarray(x[b][h::2].T).astype(bf),
                "wq": WQ.astype(bf),
                "wk": WK.astype(bf),
                "wv": WV.astype(bf),
                "cmp": cmps[h],
            }
        )
    return maps


def kernel(x, WQ, WK, WV):
    run = _get_runner()
    res = run(_core_maps(x, WQ, WK, WV))
    halves = np.asarray(res[0]).reshape(NCORE, SQ, DK)
    out = np.empty((B, S, DK), np.float32)
    for c in range(NCORE):
        b, h = c // 2, c % 2
        out[b, h::2] = halves[c]
    return out


if __name__ == "__main__":
    rng = np.random.default_rng(0)
    x = rng.standard_normal((B, S, DM), dtype=np.float32)
    sc = 1.0 / np.sqrt(DM)
    WQ = rng.standard_normal((DM, DK), dtype=np.float32) * sc
    WK = rng.standard_normal((DM, DK), dtype=np.float32) * sc
    WV = rng.standard_normal((DM, DK), dtype=np.float32) * sc
    got = kernel(x, WQ, WK, WV)
    # numpy reference
    Q = x @ WQ
    K = x @ WK
    V = x @ WV
    sref = np.einsum("bqd,bkd->bqk", Q, K) / 8.0
    mask = np.tril(np.ones((S, S), bool))
    sref = np.where(mask, sref, -np.inf)
    sref = sref - sref.max(-1, keepdims=True)
    p = np.exp(sref)
    p /= p.sum(-1, keepdims=True)
    ref = np.einsum("bqk,bkv->bqv", p, V)
    err = np.abs(got - ref).max() / np.abs(ref).max()
    print("rel err:", err)


# revision 17
# speedup vs baseline: 21.4041x; 1.2374x over previous
"""Causal single-head attention on 8 trn2 NeuronCores — single SPMD program.

Problem: x[4,4096,768], WQ/WK/WV[768,64] -> out[4,4096,64]
  Q=x@WQ K=x@WK V=x@WV; causal softmax(QK^T/8)@V per batch.

Sharding: core c handles batch b=c//2 with query-interleave h=c%2 (its
queries are global rows h, h+2, h+4, ... of the batch).  Interleaving
makes the causal structure IDENTICAL on all 8 cores: local q-block j
(512 queries spanning global rows 1024j+h .. 1024j+1022+h) always sees
k-chunks 0..8j+7.  The only h-dependence is a +-1 shift of the diagonal
mask boundary, which is supplied as a tiny per-core 0/1 mask INPUT and
applied with one vector multiply after exp — so one program serves all
cores and the whole forward pass is a single 8-device dispatch.

Host staging (outside the kernel): x is pre-transposed per core
(xt = x[b].T for K/V, xqt = x[b][h::2].T for Q), which removes all PE
transposes of x from the device kernel.

Kernel per core (all fp32):
  For each s-block sb (512 keys): DMA xT tile, project [K^T|V^T] with
  fused [WK|WV] stationary; build Vplus=[V|1] chunks via PE transpose.
  On even sb: project Q^T for q-block j=sb//2.  After odd sb: flash
  attention for q-block j=sb//2 with scores TRANSPOSED (keys on
  partitions): scoresT[k,q] = matmul(KT chunk, QT block); exp on ACT
  (scale=1/8, no max subtraction — scores ~ N(0,1), safe in fp32);
  causal zeroing on the 8 diagonal chunks via mask multiply;
  OT[65,q] += matmul(Vplus[128,65], PT) — row 64 = softmax denominator.
  Epilogue: PE-transpose OT -> O natural, reciprocal * scale, DMA out.
"""
import sys
import os

sys.path.insert(0, "/opt/trn_rl_repo")

import numpy as np

B, S, DM, DK = 4, 4096, 768, 64
NCORE = 8
SQ = S // 2          # queries per core (interleaved)
NSB = S // 512       # 8 s-blocks of 512 keys
NQB = SQ // 512      # 4 local q-blocks of 512 queries

_cache = {}


def _split_waits(nc, mybir, maxw=1):
    """Walrus here accepts only 1 sem-wait per instruction; move excess
    waits onto preceding same-engine no-ops."""
    cnt = 0
    for bb in nc.m.functions[0].blocks:
        new_insts = []
        for inst in bb.instructions:
            si = inst.sync_info
            if si is not None and si.on_wait and len(si.on_wait) > maxw:
                waits = list(si.on_wait)
                si.on_wait = waits[:maxw]
                extra = waits[maxw:]
                for i in range(0, len(extra), maxw):
                    cnt += 1
                    nop = mybir.InstNoOp(name=f"waitsplit-{cnt}", ins=[], outs=[])
                    nop.engine = inst.engine
                    nop.sync_info = mybir.SyncInfo(
                        on_wait=extra[i : i + maxw], on_update=[]
                    )
                    new_insts.append(nop)
            new_insts.append(inst)
        bb.instructions[:] = new_insts


def _build_program(reps=1):
    import concourse.bass as bass
    import concourse.mybir as mybir
    from concourse.tile import TileContext
    from concourse.masks import make_identity

    f32 = mybir.dt.float32
    bf16 = mybir.dt.bfloat16
    AF = mybir.ActivationFunctionType

    nc = bass.Bass()
    xt = nc.declare_dram_parameter("xt", [DM, S], bf16, isOutput=False)
    xqt = nc.declare_dram_parameter("xqt", [DM, SQ], bf16, isOutput=False)
    wq = nc.declare_dram_parameter("wq", [DM, DK], bf16, isOutput=False)
    wk = nc.declare_dram_parameter("wk", [DM, DK], bf16, isOutput=False)
    wv = nc.declare_dram_parameter("wv", [DM, DK], bf16, isOutput=False)
    cmp_ = nc.declare_dram_parameter("cmp", [128, 512], f32, isOutput=False)
    out = nc.declare_dram_parameter("out", [SQ, DK], f32, isOutput=True)

    with TileContext(nc) as tc:
        with (
            tc.tile_pool(name="consts", bufs=1) as cpool,
            tc.tile_pool(name="big", bufs=1) as big,
        ):
            ident = cpool.tile([128, 128], f32)
            make_identity(nc, ident[:])
            identb = cpool.tile([128, 128], bf16)
            nc.vector.tensor_copy(identb[:], ident[:])
            # [WK|WV] stationary chunks: cols 0:64 = WK, 64:128 = WV.
            # One strided DMA per weight tensor (dest viewed [128, 6, .]).
            wkv = cpool.tile([128, 6, 128], bf16)
            wqt = cpool.tile([128, 6 * 64], bf16)
            nc.sync.dma_start(
                wkv[:, :, 0:64], wk[:].rearrange("(c p) d -> p c d", p=128)
            )
            nc.sync.dma_start(
                wkv[:, :, 64:128], wv[:].rearrange("(c p) d -> p c d", p=128)
            )
            nc.sync.dma_start(
                wqt[:].rearrange("p (c d) -> p c d", d=64),
                wq[:].rearrange("(c p) d -> p c d", p=128),
            )
            cmpt = cpool.tile([128, 512], f32)
            nc.sync.dma_start(cmpt[:], cmp_[:])

            KTVT = big.tile([128, S], bf16)  # rows 0:64 = K^T, 64:128 = V^T
            QT = big.tile([64, SQ], bf16)  # own queries, transposed
            VP = big.tile([128, 32 * 65], bf16)  # [V|1] per k-chunk
            VPr = VP[:].rearrange("p (c u) -> p c u", u=65)
            nc.gpsimd.memset(VP[:], 1.0)  # ones column; V part overwritten

            # PSUM budget (8 banks): kvqt 2 + tp 2 + st 2 + ot 2
            with (
                tc.tile_pool(name="xload", bufs=2) as xl_pool,
                tc.tile_pool(name="kv_ps", bufs=2, space="PSUM") as kv_psum,
                tc.tile_pool(name="tp_ps", bufs=1, space="PSUM") as tp_psum,
                tc.tile_pool(name="s_ps", bufs=3, space="PSUM") as s_psum,
                tc.tile_pool(name="ot_ps", bufs=2, space="PSUM") as ot_psum,
                tc.tile_pool(name="pt", bufs=6) as pt_pool,
                tc.tile_pool(name="ep", bufs=2) as ep_pool,
            ):


                def emit_epilogue(j, ot):
                    ots = ep_pool.tile([65, 512], f32, name="ots", tag="ots")
                    nc.vector.tensor_copy(ots[:], ot[:])
                    o_n = ep_pool.tile([128, 4, 65], f32, name="o_n", tag="on")
                    rec = ep_pool.tile([128, 4], f32, name="rec", tag="rc")
                    for u in range(4):
                        tp2 = s_psum.tile([128, 512], f32, name="tp2", tag="st")
                        nc.tensor.transpose(
                            tp2[:, 0:65],
                            ots[:, u * 128 : (u + 1) * 128],
                            ident[0:65, 0:65],
                        )
                        nc.vector.tensor_copy(o_n[:, u, :], tp2[:, 0:65])
                        nc.vector.reciprocal(rec[:, u : u + 1], o_n[:, u, 64:65])
                        nc.vector.tensor_scalar_mul(
                            o_n[:, u, 0:64], o_n[:, u, 0:64], rec[:, u : u + 1]
                        )
                    ov = out[j * 512 : (j + 1) * 512, :].rearrange(
                        "(u p) d -> p u d", p=128
                    )
                    nc.sync.dma_start(ov, o_n[:, :, 0:64])

                for _rep in range(reps):
                    # Input staging in double-buffered 2-block tiles: one
                    # DMA per (d-chunk, block) spreads load across DMA
                    # queues; pair granularity keeps deps fine-grained and
                    # lets the next rep's DMA overlap this rep's tail.
                    xprs, xqprs = [], []
                    for pr in range(NSB // 2):
                        xpr = xl_pool.tile([128, 6, 1024], bf16, tag="xa")
                        xprs.append(xpr)
                        for half in range(2):
                            sb = pr * 2 + half
                            for c in range(6):
                                nc.sync.dma_start(
                                    xpr[:, c, half * 512 : (half + 1) * 512],
                                    xt[c * 128 : (c + 1) * 128,
                                       sb * 512 : (sb + 1) * 512],
                                )
                        if pr % 2 == 0:
                            xqpr = xl_pool.tile([128, 6, 1024], bf16, tag="xq")
                            xqprs.append(xqpr)
                            for half in range(2):
                                j = pr + half
                                for c in range(6):
                                    nc.sync.dma_start(
                                        xqpr[:, c, half * 512 : (half + 1) * 512],
                                        xqt[c * 128 : (c + 1) * 128,
                                            j * 512 : (j + 1) * 512],
                                    )
                    for sb in range(NSB):
                        # ---- K/V projection for this s-block ----
                        pass
                        kv = kv_psum.tile([128, 512], f32, name="kv", tag="kvqt")
                        for c in range(6):
                            nc.tensor.matmul(
                                kv[:],
                                wkv[:, c, :],
                                xprs[sb // 2][
                                    :, c, (sb % 2) * 512 : (sb % 2 + 1) * 512
                                ],
                                start=(c == 0),
                                stop=(c == 5),
                            )
                        nc.vector.tensor_copy(
                            KTVT[:, sb * 512 : (sb + 1) * 512], kv[:]
                        )
                        # Vplus chunks for this s-block (V natural layout)
                        for u in range(4):
                            kc = sb * 4 + u
                            vtp = tp_psum.tile([128, 64], bf16, name="vtp", tag="tp")
                            nc.tensor.transpose(
                                vtp[:, :],
                                KTVT[64:128, kc * 128 : (kc + 1) * 128],
                                identb[64:128, 64:128],
                            )
                            nc.vector.tensor_copy(VPr[:, kc, 0:64], vtp[:, :])
                        if sb % 2 == 0:
                            # ---- Q projection for q-block j = sb//2 ----
                            j = sb // 2
                            qt = kv_psum.tile([64, 512], f32, name="qt", tag="kvqt")
                            for c in range(6):
                                nc.tensor.matmul(
                                    qt[:],
                                    wqt[:, c * 64 : (c + 1) * 64],
                                    xqprs[j // 2][
                                        :, c, (j % 2) * 512 : (j % 2 + 1) * 512
                                    ],
                                    start=(c == 0),
                                    stop=(c == 5),
                                )
                            nc.vector.tensor_copy(
                                QT[:, j * 512 : (j + 1) * 512], qt[:]
                            )
                        else:
                            # ---- attention for q-block j = sb//2 ----
                            # Software-pipelined: scores/exp run L chunks
                            # ahead of the PV accumulation so the PE never
                            # waits on the ACT->mask chain.
                            j = sb // 2
                            nkc = 8 * j + 8
                            L = 2
                            ot = ot_psum.tile([65, 512], f32, name="ot", tag="ot")
                            pts = {}
                            # lo[d]: first query column any key of diagonal
                            # chunk d can see — columns below it are fully
                            # masked, so all engines skip them.
                            los = {}
                            for kc in range(nkc + L):
                                if kc < nkc:
                                    d = kc - 8 * j
                                    lo = 0 if d < 2 else 64 * (d - 1)
                                    los[kc] = lo
                                    st = s_psum.tile(
                                        [128, 512], f32, name="st", tag="st"
                                    )
                                    nc.tensor.matmul(
                                        st[:, lo:512],
                                        KTVT[0:64, kc * 128 : (kc + 1) * 128],
                                        QT[:, j * 512 + lo : (j + 1) * 512],
                                        start=True,
                                        stop=True,
                                    )
                                    pt = pt_pool.tile(
                                        [128, 512], bf16, name="pt", tag="pt"
                                    )
                                    nc.scalar.activation(
                                        pt[:, lo:512], st[:, lo:512],
                                        AF.Exp, scale=0.125
                                    )
                                    if d >= 0:
                                        # pt *= (C >= 128d): causal zeroing
                                        nc.vector.scalar_tensor_tensor(
                                            pt[:, lo:512],
                                            cmpt[:, lo:512],
                                            float(128 * d),
                                            pt[:, lo:512],
                                            op0=mybir.AluOpType.is_ge,
                                            op1=mybir.AluOpType.mult,
                                        )
                                    pts[kc] = pt
                                if kc >= L:
                                    lo = los.pop(kc - L)
                                    nc.tensor.matmul(
                                        ot[:, lo:512],
                                        VPr[:, kc - L, :],
                                        pts.pop(kc - L)[:, lo:512],
                                        start=(kc - L == 0),
                                        stop=(kc - L == nkc - 1),
                                    )
                            emit_epilogue(j, ot)

    import concourse.mybir as mybir
    _split_waits(nc, mybir)
    return nc


def _make_runner(nc, n_cores, dev_offset):
    """Compile to a jitted shard_map callable over an explicit device subset."""
    import jax
    import concourse.mybir as mybir
    from concourse import bass2jax
    from jax.experimental.shard_map import shard_map
    from jax.sharding import Mesh, PartitionSpec, NamedSharding

    bass2jax.install_neuronx_cc_hook()

    partition_name = (
        nc.partition_id_tensor.name if nc.partition_id_tensor else None
    )
    in_names, out_names, out_avals, zero_outs = [], [], [], []
    for alloc in nc.m.functions[0].allocations:
        if not isinstance(alloc, mybir.MemoryLocationSet):
            continue
        name = alloc.memorylocations[0].name
        if alloc.kind == "ExternalInput":
            if name != partition_name:
                in_names.append(name)
        elif alloc.kind == "ExternalOutput":
            shape = tuple(alloc.tensor_shape)
            dtype = mybir.dt.np(alloc.dtype)
            out_avals.append(jax.core.ShapedArray(shape, dtype))
            out_names.append(name)
            zero_outs.append(np.zeros(shape, dtype))
    n_params = len(in_names)
    n_outs = len(out_avals)
    all_names = in_names + out_names
    if partition_name is not None:
        all_names.append(partition_name)

    def _body(*args):
        operands = list(args)
        if partition_name is not None:
            operands.append(bass2jax.partition_id_tensor())
        outs = bass2jax._bass_exec_p.bind(
            *operands,
            out_avals=tuple(out_avals),
            in_names=tuple(all_names),
            out_names=tuple(out_names),
            lowering_input_output_aliases=(),
            sim_require_finite=True,
            sim_require_nnan=True,
            nc=nc,
        )
        return tuple(outs)

    devices = jax.devices()[dev_offset : dev_offset + n_cores]
    mesh = Mesh(np.asarray(devices), ("core",))
    in_specs = (PartitionSpec("core"),) * (n_params + n_outs)
    out_specs = (PartitionSpec("core"),) * n_outs
    sharded = jax.jit(
        shard_map(
            _body, mesh=mesh, in_specs=in_specs, out_specs=out_specs, check_rep=False
        ),
        keep_unused=True,
    )
    sh = NamedSharding(mesh, PartitionSpec("core"))

    def prepare(in_maps):
        per_core = [[np.asarray(m[n]) for n in in_names] for m in in_maps]
        concat_in = [
            np.concatenate([per_core[c][i] for c in range(n_cores)], axis=0)
            for i in range(n_params)
        ]
        concat_zeros = [
            np.zeros((n_cores * z.shape[0], *z.shape[1:]), z.dtype)
            for z in zero_outs
        ]
        return [jax.device_put(a, sh) for a in concat_in + concat_zeros]

    def run(in_maps):
        return sharded(*prepare(in_maps))

    run.sharded = sharded
    run.prepare = prepare
    run.out_names = out_names
    run.out_avals = out_avals
    run.n_cores = n_cores
    return run


def _bf16():
    import ml_dtypes

    return ml_dtypes.bfloat16


def _make_cmp(h):
    """Causal comparison base for interleave offset h.

    C[p, i] = 2i + h - p.  Key 128*(8j+d)+p is visible to local query
    1024j+2i+h iff C[p, i] >= 128d (j-independent), applied on-device as
    pt *= (C >= 128d) for the 8 diagonal chunks d."""
    p = np.arange(128)[:, None]
    i = np.arange(512)[None, :]
    return (2 * i + h - p).astype(np.float32)


def _get_runner(reps=1):
    key = ("runner", reps)
    if key not in _cache:
        nc = _build_program(reps)
        _cache[key] = _make_runner(nc, NCORE, 0)
    return _cache[key]


def _core_maps(x, WQ, WK, WV):
    bf = _bf16()
    cmps = [_make_cmp(0), _make_cmp(1)]
    maps = []
    for c in range(NCORE):
        b, h = c // 2, c % 2
        maps.append(
            {
                "xt": np.ascontiguousarray(x[b].T).astype(bf),
                "xqt": np.ascontiguousarray(x[b][h::2].T).astype(bf),
                "wq": WQ.astype(bf),
                "wk": WK.astype(bf),
                "wv": WV.astype(bf),
                "cmp": cmps[h],
            }
        )
    return maps


def kernel(x, WQ, WK, WV):
    run = _get_runner()
    res = run(_core_maps(x, WQ, WK, WV))
    halves = np.asarray(res[0]).reshape(NCORE, SQ, DK)
    out = np.empty((B, S, DK), np.float32)
    for c in range(NCORE):
        b, h = c // 2, c % 2
        out[b, h::2] = halves[c]
    return out


if __name__ == "__main__":
    rng = np.random.default_rng(0)
    x = rng.standard_normal((B, S, DM), dtype=np.float32)
    sc = 1.0 / np.sqrt(DM)
    WQ = rng.standard_normal((DM, DK), dtype=np.float32) * sc
    WK = rng.standard_normal((DM, DK), dtype=np.float32) * sc
    WV = rng.standard_normal((DM, DK), dtype=np.float32) * sc
    got = kernel(x, WQ, WK, WV)
    # numpy reference
    Q = x @ WQ
    K = x @ WK
    V = x @ WV
    sref = np.einsum("bqd,bkd->bqk", Q, K) / 8.0
    mask = np.tril(np.ones((S, S), bool))
    sref = np.where(mask, sref, -np.inf)
    sref = sref - sref.max(-1, keepdims=True)
    p = np.exp(sref)
    p /= p.sum(-1, keepdims=True)
    ref = np.einsum("bqk,bkv->bqv", p, V)
    err = np.abs(got - ref).max() / np.abs(ref).max()
    print("rel err:", err)


# revision 20
# speedup vs baseline: 27.2110x; 1.2713x over previous
"""Causal single-head attention on 8 trn2 NeuronCores — single SPMD program.

Problem: x[4,4096,768], WQ/WK/WV[768,64] -> out[4,4096,64]
  Q=x@WQ K=x@WK V=x@WV; causal softmax(QK^T/8)@V per batch.

Sharding: core c handles batch b=c//2 with query-interleave h=c%2 (its
queries are global rows h, h+2, h+4, ... of the batch).  Interleaving
makes the causal structure IDENTICAL on all 8 cores: local q-block j
(512 queries spanning global rows 1024j+h .. 1024j+1022+h) always sees
k-chunks 0..8j+7.  The only h-dependence is a +-1 shift of the diagonal
mask boundary, which is supplied as a tiny per-core 0/1 mask INPUT and
applied with one vector multiply after exp — so one program serves all
cores and the whole forward pass is a single 8-device dispatch.

Host staging (outside the kernel): x is pre-transposed per core
(xt = x[b].T for K/V, xqt = x[b][h::2].T for Q), which removes all PE
transposes of x from the device kernel.

Kernel per core (all fp32):
  For each s-block sb (512 keys): DMA xT tile, project [K^T|V^T] with
  fused [WK|WV] stationary; build Vplus=[V|1] chunks via PE transpose.
  On even sb: project Q^T for q-block j=sb//2.  After odd sb: flash
  attention for q-block j=sb//2 with scores TRANSPOSED (keys on
  partitions): scoresT[k,q] = matmul(KT chunk, QT block); exp on ACT
  (scale=1/8, no max subtraction — scores ~ N(0,1), safe in fp32);
  causal zeroing on the 8 diagonal chunks via mask multiply;
  OT[65,q] += matmul(Vplus[128,65], PT) — row 64 = softmax denominator.
  Epilogue: PE-transpose OT -> O natural, reciprocal * scale, DMA out.
"""
import sys
import os

sys.path.insert(0, "/opt/trn_rl_repo")

import numpy as np

B, S, DM, DK = 4, 4096, 768, 64
NCORE = 8
SQ = S // 2          # queries per core (interleaved)
NSB = S // 512       # 8 s-blocks of 512 keys
NQB = SQ // 512      # 4 local q-blocks of 512 queries

_cache = {}


def _split_waits(nc, mybir, maxw=1):
    """Walrus here accepts only 1 sem-wait per instruction; move excess
    waits onto preceding same-engine no-ops."""
    cnt = 0
    for bb in nc.m.functions[0].blocks:
        new_insts = []
        for inst in bb.instructions:
            si = inst.sync_info
            if si is not None and si.on_wait and len(si.on_wait) > maxw:
                waits = list(si.on_wait)
                si.on_wait = waits[:maxw]
                extra = waits[maxw:]
                for i in range(0, len(extra), maxw):
                    cnt += 1
                    nop = mybir.InstNoOp(name=f"waitsplit-{cnt}", ins=[], outs=[])
                    nop.engine = inst.engine
                    nop.sync_info = mybir.SyncInfo(
                        on_wait=extra[i : i + maxw], on_update=[]
                    )
                    new_insts.append(nop)
            new_insts.append(inst)
        bb.instructions[:] = new_insts


def _build_program(reps=1):
    import concourse.bass as bass
    import concourse.mybir as mybir
    from concourse.tile import TileContext
    from concourse.masks import make_identity

    f32 = mybir.dt.float32
    bf16 = mybir.dt.bfloat16
    AF = mybir.ActivationFunctionType

    nc = bass.Bass()
    xt = nc.declare_dram_parameter("xt", [DM, S // 2], bf16, isOutput=False)
    xqt = nc.declare_dram_parameter("xqt", [DM, SQ], bf16, isOutput=False)
    wq = nc.declare_dram_parameter("wq", [DM, DK], bf16, isOutput=False)
    wk = nc.declare_dram_parameter("wk", [DM, DK], bf16, isOutput=False)
    wv = nc.declare_dram_parameter("wv", [DM, DK], bf16, isOutput=False)
    cmp_ = nc.declare_dram_parameter("cmp", [128, 512], f32, isOutput=False)
    out = nc.declare_dram_parameter("out", [SQ, DK], f32, isOutput=True)

    with TileContext(nc) as tc:
        with (
            tc.tile_pool(name="consts", bufs=1) as cpool,
            tc.tile_pool(name="big", bufs=1) as big,
        ):
            ident = cpool.tile([128, 128], f32)
            make_identity(nc, ident[:])
            identb = cpool.tile([128, 128], bf16)
            nc.vector.tensor_copy(identb[:], ident[:])
            # [WK|WV] stationary chunks: cols 0:64 = WK, 64:128 = WV.
            # One strided DMA per weight tensor (dest viewed [128, 6, .]).
            wkv = cpool.tile([128, 6, 128], bf16)
            wqt = cpool.tile([128, 6 * 64], bf16)
            nc.sync.dma_start(
                wkv[:, :, 0:64], wk[:].rearrange("(c p) d -> p c d", p=128)
            )
            nc.sync.dma_start(
                wkv[:, :, 64:128], wv[:].rearrange("(c p) d -> p c d", p=128)
            )
            nc.sync.dma_start(
                wqt[:].rearrange("p (c d) -> p c d", d=64),
                wq[:].rearrange("(c p) d -> p c d", p=128),
            )
            cmpt = cpool.tile([128, 512], f32)
            nc.sync.dma_start(cmpt[:], cmp_[:])

            KTVT = big.tile([128, S], bf16)  # rows 0:64 = K^T, 64:128 = V^T
            QT = big.tile([64, SQ], bf16)  # own queries, transposed
            VP = big.tile([128, 32 * 65], bf16)  # [V|1] per k-chunk
            VPr = VP[:].rearrange("p (c u) -> p c u", u=65)
            nc.gpsimd.memset(VP[:], 1.0)  # ones column; V part overwritten

            # PSUM budget (8 banks): kvqt 2 + tp 1 + st 3 + ot 2
            with (
                tc.tile_pool(name="xload", bufs=2) as xl_pool,
                tc.tile_pool(name="kvs", bufs=2) as kvs_pool,
                tc.tile_pool(name="ccd", bufs=2, space="DRAM") as dram_pool,
                tc.tile_pool(name="kv_ps", bufs=2, space="PSUM") as kv_psum,
                tc.tile_pool(name="tp_ps", bufs=1, space="PSUM") as tp_psum,
                tc.tile_pool(name="s_ps", bufs=3, space="PSUM") as s_psum,
                tc.tile_pool(name="ot_ps", bufs=2, space="PSUM") as ot_psum,
                tc.tile_pool(name="pt", bufs=6) as pt_pool,
                tc.tile_pool(name="ep", bufs=2) as ep_pool,
            ):


                def emit_epilogue(j, ot):
                    ots = ep_pool.tile([65, 512], f32, name="ots", tag="ots")
                    nc.vector.tensor_copy(ots[:], ot[:])
                    o_n = ep_pool.tile([128, 4, 65], f32, name="o_n", tag="on")
                    rec = ep_pool.tile([128, 4], f32, name="rec", tag="rc")
                    for u in range(4):
                        tp2 = s_psum.tile([128, 512], f32, name="tp2", tag="st")
                        nc.tensor.transpose(
                            tp2[:, 0:65],
                            ots[:, u * 128 : (u + 1) * 128],
                            ident[0:65, 0:65],
                        )
                        nc.vector.tensor_copy(o_n[:, u, :], tp2[:, 0:65])
                        nc.vector.reciprocal(rec[:, u : u + 1], o_n[:, u, 64:65])
                        nc.vector.tensor_scalar_mul(
                            o_n[:, u, 0:64], o_n[:, u, 0:64], rec[:, u : u + 1]
                        )
                    ov = out[j * 512 : (j + 1) * 512, :].rearrange(
                        "(u p) d -> p u d", p=128
                    )
                    nc.sync.dma_start(ov, o_n[:, :, 0:64])

                for _rep in range(reps):
                    # Input staging: each core holds only its OWN 4
                    # s-blocks (host permutes so slot s = global block
                    # 2s+rank%2); K/V of the partner block arrives via a
                    # 2-core AllGather below.  One DMA per (d-chunk, slot)
                    # spreads load across DMA queues.
                    xprs, xqprs = [], []
                    for sl in range(4):
                        xpr = xl_pool.tile([128, 6, 512], bf16, tag="xa")
                        xprs.append(xpr)
                        for c in range(6):
                            nc.sync.dma_start(
                                xpr[:, c, :],
                                xt[c * 128 : (c + 1) * 128,
                                   sl * 512 : (sl + 1) * 512],
                            )
                        if sl % 2 == 0:
                            xqpr = xl_pool.tile([128, 6, 1024], bf16, tag="xq")
                            xqprs.append(xqpr)
                            for half in range(2):
                                j = sl + half
                                for c in range(6):
                                    nc.sync.dma_start(
                                        xqpr[:, c, half * 512 : (half + 1) * 512],
                                        xqt[c * 128 : (c + 1) * 128,
                                            j * 512 : (j + 1) * 512],
                                    )
                    for sb in range(NSB):
                        if sb % 2 == 0:
                            # ---- stage s: project OWN block (global
                            # 2s+rank%2), allgather the K/V pair ----
                            s_ = sb // 2
                            kv = kv_psum.tile([128, 512], f32, name="kv", tag="kvqt")
                            for c in range(6):
                                nc.tensor.matmul(
                                    kv[:],
                                    wkv[:, c, :],
                                    xprs[s_][:, c, :],
                                    start=(c == 0),
                                    stop=(c == 5),
                                )
                            kvs = kvs_pool.tile(
                                [128, 512], bf16, name="kvs", tag="kvs"
                            )
                            nc.vector.tensor_copy(kvs[:], kv[:])
                            cc_in = dram_pool.tile([128, 512], bf16)
                            cc_out = dram_pool.tile([2, 128, 512], bf16)
                            nc.gpsimd.dma_start(cc_in[:], kvs[:])
                            nc.gpsimd.collective_compute(
                                "AllGather",
                                mybir.AluOpType.bypass,
                                replica_groups=[[0, 1], [2, 3], [4, 5], [6, 7]],
                                ins=[cc_in[:].opt()],
                                outs=[cc_out[:].opt()],
                            )
                            nc.gpsimd.dma_start(
                                KTVT[:, s_ * 1024 : (s_ + 1) * 1024].rearrange(
                                    "p (b s) -> p b s", b=2
                                ),
                                cc_out[:].rearrange("b p s -> p b s"),
                            )
                            # Vplus chunks for the gathered pair
                            for u in range(8):
                                kc = s_ * 8 + u
                                vtp = tp_psum.tile(
                                    [128, 64], bf16, name="vtp", tag="tp"
                                )
                                nc.tensor.transpose(
                                    vtp[:, :],
                                    KTVT[64:128, kc * 128 : (kc + 1) * 128],
                                    identb[64:128, 64:128],
                                )
                                nc.vector.tensor_copy(VPr[:, kc, 0:64], vtp[:, :])
                            # ---- Q projection for q-block j = sb//2 ----
                            j = sb // 2
                            qt = kv_psum.tile([64, 512], f32, name="qt", tag="kvqt")
                            for c in range(6):
                                nc.tensor.matmul(
                                    qt[:],
                                    wqt[:, c * 64 : (c + 1) * 64],
                                    xqprs[j // 2][
                                        :, c, (j % 2) * 512 : (j % 2 + 1) * 512
                                    ],
                                    start=(c == 0),
                                    stop=(c == 5),
                                )
                            nc.vector.tensor_copy(
                                QT[:, j * 512 : (j + 1) * 512], qt[:]
                            )
                        else:
                            # ---- attention for q-block j = sb//2 ----
                            # Software-pipelined: scores/exp run L chunks
                            # ahead of the PV accumulation so the PE never
                            # waits on the ACT->mask chain.
                            j = sb // 2
                            nkc = 8 * j + 8
                            L = 2
                            ot = ot_psum.tile([65, 512], f32, name="ot", tag="ot")
                            pts = {}
                            # lo[d]: first query column any key of diagonal
                            # chunk d can see — columns below it are fully
                            # masked, so all engines skip them.
                            los = {}
                            for kc in range(nkc + L):
                                if kc < nkc:
                                    d = kc - 8 * j
                                    lo = 0 if d < 2 else 64 * (d - 1)
                                    los[kc] = lo
                                    st = s_psum.tile(
                                        [128, 512], f32, name="st", tag="st"
                                    )
                                    nc.tensor.matmul(
                                        st[:, lo:512],
                                        KTVT[0:64, kc * 128 : (kc + 1) * 128],
                                        QT[:, j * 512 + lo : (j + 1) * 512],
                                        start=True,
                                        stop=True,
                                    )
                                    pt = pt_pool.tile(
                                        [128, 512], bf16, name="pt", tag="pt"
                                    )
                                    nc.scalar.activation(
                                        pt[:, lo:512], st[:, lo:512],
                                        AF.Exp, scale=0.125
                                    )
                                    if d >= 0:
                                        # pt *= (C >= 128d): causal zeroing
                                        nc.vector.scalar_tensor_tensor(
                                            pt[:, lo:512],
                                            cmpt[:, lo:512],
                                            float(128 * d),
                                            pt[:, lo:512],
                                            op0=mybir.AluOpType.is_ge,
                                            op1=mybir.AluOpType.mult,
                                        )
                                    pts[kc] = pt
                                if kc >= L:
                                    lo = los.pop(kc - L)
                                    nc.tensor.matmul(
                                        ot[:, lo:512],
                                        VPr[:, kc - L, :],
                                        pts.pop(kc - L)[:, lo:512],
                                        start=(kc - L == 0),
                                        stop=(kc - L == nkc - 1),
                                    )
                            emit_epilogue(j, ot)

    import concourse.mybir as mybir
    _split_waits(nc, mybir)
    return nc


def _make_runner(nc, n_cores, dev_offset):
    """Compile to a jitted shard_map callable over an explicit device subset."""
    import jax
    import concourse.mybir as mybir
    from concourse import bass2jax
    from jax.experimental.shard_map import shard_map
    from jax.sharding import Mesh, PartitionSpec, NamedSharding

    bass2jax.install_neuronx_cc_hook()

    partition_name = (
        nc.partition_id_tensor.name if nc.partition_id_tensor else None
    )
    in_names, out_names, out_avals, zero_outs = [], [], [], []
    for alloc in nc.m.functions[0].allocations:
        if not isinstance(alloc, mybir.MemoryLocationSet):
            continue
        name = alloc.memorylocations[0].name
        if alloc.kind == "ExternalInput":
            if name != partition_name:
                in_names.append(name)
        elif alloc.kind == "ExternalOutput":
            shape = tuple(alloc.tensor_shape)
            dtype = mybir.dt.np(alloc.dtype)
            out_avals.append(jax.core.ShapedArray(shape, dtype))
            out_names.append(name)
            zero_outs.append(np.zeros(shape, dtype))
    n_params = len(in_names)
    n_outs = len(out_avals)
    all_names = in_names + out_names
    if partition_name is not None:
        all_names.append(partition_name)

    def _body(*args):
        operands = list(args)
        if partition_name is not None:
            operands.append(bass2jax.partition_id_tensor())
        outs = bass2jax._bass_exec_p.bind(
            *operands,
            out_avals=tuple(out_avals),
            in_names=tuple(all_names),
            out_names=tuple(out_names),
            lowering_input_output_aliases=(),
            sim_require_finite=True,
            sim_require_nnan=True,
            nc=nc,
        )
        return tuple(outs)

    devices = jax.devices()[dev_offset : dev_offset + n_cores]
    mesh = Mesh(np.asarray(devices), ("core",))
    in_specs = (PartitionSpec("core"),) * (n_params + n_outs)
    out_specs = (PartitionSpec("core"),) * n_outs
    sharded = jax.jit(
        shard_map(
            _body, mesh=mesh, in_specs=in_specs, out_specs=out_specs, check_rep=False
        ),
        keep_unused=True,
    )
    sh = NamedSharding(mesh, PartitionSpec("core"))

    def prepare(in_maps):
        per_core = [[np.asarray(m[n]) for n in in_names] for m in in_maps]
        concat_in = [
            np.concatenate([per_core[c][i] for c in range(n_cores)], axis=0)
            for i in range(n_params)
        ]
        concat_zeros = [
            np.zeros((n_cores * z.shape[0], *z.shape[1:]), z.dtype)
            for z in zero_outs
        ]
        return [jax.device_put(a, sh) for a in concat_in + concat_zeros]

    def run(in_maps):
        return sharded(*prepare(in_maps))

    run.sharded = sharded
    run.prepare = prepare
    run.out_names = out_names
    run.out_avals = out_avals
    run.n_cores = n_cores
    return run


def _bf16():
    import ml_dtypes

    return ml_dtypes.bfloat16


def _make_cmp(h):
    """Causal comparison base for interleave offset h.

    C[p, i] = 2i + h - p.  Key 128*(8j+d)+p is visible to local query
    1024j+2i+h iff C[p, i] >= 128d (j-independent), applied on-device as
    pt *= (C >= 128d) for the 8 diagonal chunks d."""
    p = np.arange(128)[:, None]
    i = np.arange(512)[None, :]
    return (2 * i + h - p).astype(np.float32)


def _get_runner(reps=1):
    key = ("runner", reps)
    if key not in _cache:
        nc = _build_program(reps)
        _cache[key] = _make_runner(nc, NCORE, 0)
    return _cache[key]


def _core_maps(x, WQ, WK, WV):
    bf = _bf16()
    cmps = [_make_cmp(0), _make_cmp(1)]
    maps = []
    for c in range(NCORE):
        b, h = c // 2, c % 2
        maps.append(
            {
                "xt": np.ascontiguousarray(
                    x[b].T.reshape(768, 8, 512)[:, h::2, :].reshape(768, 2048)
                ).astype(bf),
                "xqt": np.ascontiguousarray(x[b][h::2].T).astype(bf),
                "wq": WQ.astype(bf),
                "wk": WK.astype(bf),
                "wv": WV.astype(bf),
                "cmp": cmps[h],
            }
        )
    return maps


def kernel(x, WQ, WK, WV):
    run = _get_runner()
    res = run(_core_maps(x, WQ, WK, WV))
    halves = np.asarray(res[0]).reshape(NCORE, SQ, DK)
    out = np.empty((B, S, DK), np.float32)
    for c in range(NCORE):
        b, h = c // 2, c % 2
        out[b, h::2] = halves[c]
    return out


if __name__ == "__main__":
    rng = np.random.default_rng(0)
    x = rng.standard_normal((B, S, DM), dtype=np.float32)
    sc = 1.0 / np.sqrt(DM)
    WQ = rng.standard_normal((DM, DK), dtype=np.float32) * sc
    WK = rng.standard_normal((DM, DK), dtype=np.float32) * sc
    WV = rng.standard_normal((DM, DK), dtype=np.float32) * sc
    got = kernel(x, WQ, WK, WV)
    # numpy reference
    Q = x @ WQ
    K = x @ WK
    V = x @ WV
    sref = np.einsum("bqd,bkd->bqk", Q, K) / 8.0
    mask = np.tril(np.ones((S, S), bool))
    sref = np.where(mask, sref, -np.inf)
    sref = sref - sref.max(-1, keepdims=True)
    p = np.exp(sref)
    p /= p.sum(-1, keepdims=True)
    ref = np.einsum("bqk,bkv->bqv", p, V)
    err = np.abs(got - ref).max() / np.abs(ref).max()
    print("rel err:", err)
